# revision 1
# baseline (speedup 1.0000x reference)
"""Causal multi-head attention (B=1, S=4096, E=1024, H=16, Dk=64) on 8 TRN2
NeuronCores via Bass/Tile, head-sharded (tensor parallel): core c computes
heads 2c and 2c+1 end-to-end plus its partial output projection; the host sums
the 8 partials and adds the output bias.

Per-core program:
  QT/KT[e'=128, S] = (W x^T + b) in bf16 (softmax 1/sqrt(Dk) folded into Wq/bq)
  V'[k, 2*65]      = x Wv^T + bv, with a ones column per head
  per q-block (512) x k-tile (128, causal):
    scoresT[k, q] via PE (2 heads packed with row tiling, d=64 each)
    pT = exp(scoresT) on ScalarE (no max subtraction; scores are ~N(0,1))
    diagonal tiles: multiply by causal 0/1 mask strip (post-exp)
    acc_h[65, q] += V'_h.T @ pT_h   (row 64 accumulates the softmax denom)
  attn_cT = acc[0:64] * broadcast(1/acc[64]);  partial = attn_cT.T @ Wo_c.T
"""

import numpy as np

import concourse.bass as bass
import concourse.mybir as mybir
import concourse.tile as tile
from concourse import bacc
from concourse.bass_utils import run_bass_kernel_spmd

F32 = mybir.dt.float32
BF16 = mybir.dt.bfloat16
AF = mybir.ActivationFunctionType

EMBED_DIM = 1024
NUM_HEADS = 16
SEQ = 4096
BATCH = 1
N_CORES = 8


def _build_nc(S=SEQ, E=EMBED_DIM):
    EC = 128          # per-core feature slice (2 heads x 64)
    NI = E // 128     # contraction tiles for projections
    NQB = S // 512    # q blocks
    NKT = S // 128    # k tiles

    nc = bacc.Bacc(None, target_bir_lowering=False, debug=False)

    # x arrives pre-permuted to the SBUF layout: xP[p, sb, it, s'] =
    # x[sb*512+s', it*128+p] -- one contiguous 8KB line per partition per
    # 512-column s-block (full-rate DMA, no mid-dim segmentation)
    xP = nc.dram_tensor("xP", [128, S // 512, E // 128, 512], BF16,
                        kind="ExternalInput")
    # projection weights arrive pre-packed as [128, NI*EC]:
    # packed[p, it*EC + e] = W.T[it*128 + p, e]  (contiguous DMA lines)
    wqT = nc.dram_tensor("wqT", [128, NI * EC], BF16, kind="ExternalInput")
    wkT = nc.dram_tensor("wkT", [128, NI * EC], BF16, kind="ExternalInput")
    wvT = nc.dram_tensor("wvT", [128, NI * EC], BF16, kind="ExternalInput")
    woT = nc.dram_tensor("woT", [EC, E], BF16, kind="ExternalInput")
    bq = nc.dram_tensor("bq", [EC, 1], F32, kind="ExternalInput")
    bk = nc.dram_tensor("bk", [EC, 1], F32, kind="ExternalInput")
    bv = nc.dram_tensor("bv", [1, EC], F32, kind="ExternalInput")
    maskst = nc.dram_tensor("maskst", [128, 896], BF16, kind="ExternalInput")
    out = nc.dram_tensor("out", [S, E], F32, kind="ExternalOutput")

    with tile.TileContext(nc) as tc:
        with tc.tile_pool(name="const", bufs=1) as const:
            # q/k weights + small constants first, then x (the long pole),
            # then v/o weights (not needed until attention starts)
            w_sb = {}
            for name, wt in (("q", wqT), ("k", wkT), ("v", wvT)):
                w_sb[name] = const.tile([128, NI, EC], BF16, tag=f"w{name}",
                                        name=f"w{name}")
            for name, wt in (("q", wqT), ("k", wkT)):
                nc.sync.dma_start(
                    out=w_sb[name][:, :, :],
                    in_=wt.ap().rearrange("p (t e) -> p t e", t=NI))

            # x streamed in s-block-major chunks: the first 1 MiB (s-block 0,
            # all 8 i-tiles) lands early so projections/attention start
            # early; later s-blocks stream behind attention demand.
            # s-block-major to mirror xP: contiguous 8KB DMA lines/partition
            xt_sb = const.tile([128, S // 512, NI, 512], BF16, tag="xt")
            bq_sb = const.tile([128, 1], F32, tag="bq")
            bk_sb = const.tile([128, 1], F32, tag="bk")
            bv_row = const.tile([1, EC], F32, tag="bvr")
            bv_bc = const.tile([128, EC], F32, tag="bv")
            mask_sb = const.tile([128, 896], BF16, tag="mask")
            wo_sb = const.tile([128, E], BF16, tag="wo")
            for sb in range(S // 512):
                nc.sync.dma_start(out=xt_sb[:, sb, :, :], in_=xP[:, sb, :, :])
                if sb == 0:
                    nc.sync.dma_start(out=bq_sb, in_=bq[:, :])
                    nc.sync.dma_start(out=bk_sb, in_=bk[:, :])
                    nc.sync.dma_start(out=bv_row, in_=bv[:, :])
                    nc.gpsimd.partition_broadcast(bv_bc[:, :], bv_row[0:1, :])
                    nc.sync.dma_start(out=mask_sb, in_=maskst[:, :])
                    nc.sync.dma_start(
                        out=w_sb["v"][:, :, :],
                        in_=wvT.ap().rearrange("p (t e) -> p t e", t=NI))
                elif sb == 1:
                    nc.sync.dma_start(out=wo_sb, in_=woT[:, :])

            qt_sb = const.tile([128, S], BF16, tag="qt")
            kt_sb = const.tile([128, S], BF16, tag="kt")
            v_sb = const.tile([128, NKT, 130], BF16, tag="v")
            nc.vector.memset(v_sb[:, :, 64:65], 1.0)
            nc.vector.memset(v_sb[:, :, 129:130], 1.0)

            # single PSUM pool; tags shared across phases so banks flow from
            # projections into attention without a phase barrier.
            # banks: sc 2x2 + acc0/acc1 1x1 each + op 2x1 = 8
            with tc.tile_pool(name="ps", bufs=1, space="PSUM") as ps_pool, \
                 tc.tile_pool(name="spt", bufs=13) as spt, \
                 tc.tile_pool(name="sat", bufs=9) as sat, \
                 tc.tile_pool(name="sdiv", bufs=6) as sdiv, \
                 tc.tile_pool(name="sout", bufs=12) as sout:

                def emit_qkproj_one(name, dst, bias, sb):
                    w = w_sb[name]
                    ps = ps_pool.tile([128, 1024], F32, tag="sc", bufs=2,
                                      name=f"pj{name}{sb}")
                    for it in range(NI):
                        nc.tensor.matmul(
                            ps[:, 0:512],
                            lhsT=w[:, it, :],
                            rhs=xt_sb[:, sb, it, :],
                            start=(it == 0), stop=(it == NI - 1),
                        )
                    nc.vector.tensor_scalar_add(
                        dst[:, sb * 512:(sb + 1) * 512], ps[:, 0:512],
                        bias[:, 0:1])

                def emit_qkproj(sb):
                    emit_qkproj_one("q", qt_sb, bq_sb, sb)
                    emit_qkproj_one("k", kt_sb, bk_sb, sb)

                wv = w_sb["v"]
                vproj_done = [0]

                def emit_vproj_one(st):
                    ps = ps_pool.tile([128, 512], F32, tag="op", bufs=2,
                                      name=f"pjv{st}")
                    for it in range(NI):
                        nc.tensor.matmul(
                            ps[:, 0:EC],
                            lhsT=xt_sb[:, st // 4, it,
                                       (st % 4) * 128:(st % 4) * 128 + 128],
                            rhs=wv[:, it, :],
                            start=(it == 0), stop=(it == NI - 1),
                        )
                    nc.vector.tensor_add(
                        v_sb[:, st, 0:64], ps[:, 0:64], bv_bc[:, 0:64])
                    nc.vector.tensor_add(
                        v_sb[:, st, 65:129], ps[:, 64:128], bv_bc[:, 64:128])

                def emit_vproj(upto):
                    for st in range(vproj_done[0], min(upto, NKT)):
                        emit_vproj_one(st)
                    vproj_done[0] = max(vproj_done[0], min(upto, NKT))

                def emit_attnv(acc, jpt, nkt):
                    j, pt, off, w = jpt
                    for h in range(2):
                        nc.tensor.matmul(
                            acc[h][:, off:512],
                            lhsT=v_sb[:, j, 65 * h:65 * h + 65],
                            rhs=pt[:, 512 * h:512 * h + w],
                            start=(j == 0), stop=(j == nkt - 1),
                        )

                pending_oproj = []
                pending_epi = []

                # HAM warmup: run throwaway matmuls while the first DMAs are
                # in flight so the real projections start at the warm clock.
                warm_src = const.tile([128, 512], BF16, tag="warmsrc")
                nc.vector.memset(warm_src[:, :], 1.0)
                for i in range(14):
                    wp = ps_pool.tile([128, 1024], F32, tag="sc", bufs=2,
                                      name=f"warm{i}")
                    nc.tensor.matmul(wp[:, 0:512], lhsT=warm_src[:, 0:128],
                                     rhs=warm_src[:, :], start=True, stop=True)

                emit_qkproj(0)
                emit_vproj(4)
                for qb in range(NQB):
                    # lookahead projections for qb+1, spread through the k-loop
                    # so they soak PE slack instead of stalling the exp stream
                    bg = []
                    if qb + 1 < NQB:
                        for name, dst, bias in (("q", qt_sb, bq_sb),
                                                ("k", kt_sb, bk_sb)):
                            bg.append(lambda n=name, d=dst, b=bias, s=qb + 1:
                                      emit_qkproj_one(n, d, b, s))
                    lo, hi = vproj_done[0], min(4 * (qb + 2), NKT)
                    for st in range(lo, hi):
                        bg.append(lambda st=st: emit_vproj_one(st))
                    vproj_done[0] = hi

                    nkt = 4 * (qb + 1)
                    acc = [ps_pool.tile([65, 512], F32, tag=f"acc{h}",
                                        name=f"acc{h}_{qb}")
                           for h in range(2)]
                    pts = []
                    for j in range(nkt):
                        if j == 3 and pending_epi:
                            # previous block's attnV backlog + division,
                            # deferred past this block's first score tiles
                            pending_epi.pop(0)()
                        elif j >= 8 and j % 2 == 0 and pending_oproj:
                            # previous q-block's output projection, one tile
                            # per k-tile so its PSUM rotation never stalls PE
                            pending_oproj.pop(0)()
                        elif j >= 2 and bg:
                            bg.pop(0)()
                        r = j - 4 * qb  # >= 0 on the causal diagonal
                        off = 128 * r if r > 0 else 0
                        w = 512 - off   # valid q columns for this k-tile
                        sc = ps_pool.tile([128, 1024], F32, tag="sc", bufs=2,
                                          name=f"sc{qb}_{j}")
                        for h in range(2):
                            hp = slice(64 * h, 64 * h + 64)
                            nc.tensor.matmul(
                                sc[:, 512 * h:512 * h + w],
                                lhsT=kt_sb[hp, j * 128:(j + 1) * 128],
                                rhs=qt_sb[hp, qb * 512 + off:(qb + 1) * 512],
                                start=True, stop=True,
                            )
                        pt = spt.tile([128, 1024], BF16, tag="pt", name=f"pt{qb}_{j}")
                        if r >= 0:
                            # one exp over both heads' [0:w] and [512:512+w]
                            # slices via a strided AP (keeps PSUM banks aligned)
                            def _two(t, w=w):
                                a = t[:, :]
                                return bass.AP(tensor=a.tensor, offset=a.offset,
                                               ap=[a.ap[0], [512, 2], [1, w]])
                            nc.scalar.activation(_two(pt), _two(sc), AF.Exp)
                            m = mask_sb[:, 384:384 + w]
                            for h in range(2):
                                pslc = pt[:, 512 * h:512 * h + w]
                                nc.vector.tensor_mul(pslc, pslc, m)
                        else:
                            nc.scalar.activation(pt[:, :], sc[:, :], AF.Exp)
                        pts.append((j, pt, off, w))
                        if len(pts) >= 8:
                            emit_attnv(acc, pts.pop(0), nkt)
                    while bg:  # anything not soaked up mid-loop
                        bg.pop(0)()

                    last = qb == NQB - 1

                    def emit_epi(qb=qb, acc=acc, pts=pts, nkt=nkt, last=last):
                        while pts:
                            emit_attnv(acc, pts.pop(0), nkt)
                        att = sat.tile([128, 512], BF16, tag="att",
                                       name=f"att{qb}")
                        for h in range(2):
                            rc = sdiv.tile([1, 512], F32, tag=f"rc{h}",
                                           name=f"rc{h}_{qb}")
                            nc.vector.reciprocal(rc[:, :], acc[h][64:65, :])
                            rbc = sdiv.tile([64, 512], F32, tag=f"rbc{h}",
                                            name=f"rbc{h}_{qb}")
                            nc.gpsimd.partition_broadcast(rbc[:, :], rc[0:1, :])
                            nc.vector.tensor_mul(
                                att[64 * h:64 * h + 64, :], acc[h][0:64, :],
                                rbc[:, :])
                        for st in range(4):
                            for nh in range(E // 512):
                                pending_oproj.append(
                                    lambda st=st, nh=nh, a=att, q=qb, l=last:
                                    emit_oproj_one(st, nh, qb=q, att=a, last=l))

                    pending_epi.append(emit_epi)

                    def emit_oproj_one(st, nh, qb=qb, att=None, last=last):
                        op = ps_pool.tile([128, 512], F32, tag="op", bufs=2,
                                          name=f"op{qb}_{st}_{nh}")
                        nc.tensor.matmul(
                            op[:, :],
                            lhsT=att[:, st * 128:(st + 1) * 128],
                            rhs=wo_sb[:, nh * 512:(nh + 1) * 512],
                            start=True, stop=True,
                        )
                        ob = sout.tile([128, 512], F32, tag="ob",
                                       name=f"ob{qb}_{st}_{nh}")
                        if last and (st * 2 + nh) % 2 == 0:
                            # ScalarE is idle in the kernel tail; split the
                            # PSUM drain across both engines
                            nc.scalar.copy(ob[:, :], op[:, :])
                        else:
                            nc.vector.tensor_copy(ob[:, :], op[:, :])
                        nc.sync.dma_start(
                            out=out[qb * 512 + st * 128:
                                    qb * 512 + (st + 1) * 128,
                                    nh * 512:(nh + 1) * 512],
                            in_=ob[:, :])

                while pending_epi:
                    pending_epi.pop(0)()
                while pending_oproj:
                    pending_oproj.pop(0)()

    nc.compile()
    return nc


def _make_mask_strip():
    k = np.arange(128)[:, None]
    t = np.arange(896)[None, :]
    return (k <= t - 384).astype(np.float32)


def _pack_w(wT):
    # [E, EC] -> [128, NI*EC] with packed[p, it*EC+e] = wT[it*128+p, e]
    E, EC = wT.shape
    return np.ascontiguousarray(
        wT.reshape(E // 128, 128, EC).transpose(1, 0, 2).reshape(128, -1))


def _shard_inputs(x, Wq, bq, Wk, bk, Wv, bv, Wo):
    import ml_dtypes
    bf16 = ml_dtypes.bfloat16
    S, E = x.shape[-2], x.shape[-1]
    xP = np.ascontiguousarray(
        np.asarray(x, np.float32).reshape(S // 512, 512, E // 128, 128)
        .transpose(3, 0, 2, 1)).astype(bf16)
    strip = _make_mask_strip().astype(bf16)
    in_maps = []
    for c in range(N_CORES):
        sl = slice(128 * c, 128 * (c + 1))
        in_maps.append({
            "xP": xP,
            "wqT": _pack_w((np.asarray(Wq, np.float32)[sl, :] / 8.0).T).astype(bf16),
            "wkT": _pack_w(np.asarray(Wk, np.float32)[sl, :].T).astype(bf16),
            "wvT": _pack_w(np.asarray(Wv, np.float32)[sl, :].T).astype(bf16),
            "woT": np.ascontiguousarray(np.asarray(Wo, np.float32)[:, sl].T).astype(bf16),
            "bq": (np.asarray(bq, np.float32)[sl] / 8.0).reshape(128, 1),
            "bk": np.asarray(bk, np.float32)[sl].reshape(128, 1),
            "bv": np.asarray(bv, np.float32)[sl].reshape(1, 128),
            "maskst": strip,
        })
    return in_maps


_NC_CACHE = {}


def kernel(x, Wq, bq, Wk, bk, Wv, bv, Wo, bo):
    x = np.asarray(x)
    B, S, E = x.shape
    if (S, E) not in _NC_CACHE:
        _NC_CACHE[(S, E)] = _build_nc(S=S, E=E)
    nc = _NC_CACHE[(S, E)]

    in_maps = _shard_inputs(x, Wq, bq, Wk, bk, Wv, bv, Wo)
    res = run_bass_kernel_spmd(nc, in_maps, list(range(N_CORES)))

    total = np.zeros((S, E), np.float32)
    for r in res.results:
        total += r["out"]
    total += np.asarray(bo, np.float32)
    return total.reshape(B, S, E).astype(np.float32)



# revision 88
# speedup vs baseline: 1.1165x; 1.1165x over previous
"""Causal multi-head attention (B=1, S=4096, E=1024, H=16, Dk=64) on 8 TRN2
NeuronCores via Bass/Tile, head-sharded (tensor parallel): core c computes
heads 2c and 2c+1 end-to-end plus its partial output projection; the host sums
the 8 partials (bf16) and adds the output bias.

Per-core program (transposed attn.V + global exp-ahead pipeline):
  QT/KT[e'=128, S] = (W x^T + b) in bf16 (softmax 1/sqrt(Dk) folded into Wq/bq)
  V'[k, 2*65]      = x Wv^T + bv, with a ones column per head
  global tile stream, scores->exp running AHEAD of attn.V consumption:
    scoresT[k, q] via PE (2 heads packed with row tiling, d=64 each)
    pT = exp(scoresT) on ScalarE (no max subtraction; scores are ~N(0,1))
    diagonal tiles: multiply by causal 0/1 mask strip (post-exp)
    per q-subtile (128) and head: accT_h[q, 0:65] += pT_h.T @ V'_h
      (out partitions = 128 q values -> half the PE cycles of the
       [65, q] orientation; column 64 accumulates the softmax denom)
  att[q, d] = accT[q, 0:64] * (1/accT[q, 64])   (per-partition scalar on DVE)
  attT[d, q] via PE transpose (identity matmul), then
  partial[q, e] = attT.T @ Wo_c.T ; drained to bf16 partial output
"""

import numpy as np

import concourse.bass as bass
import concourse.mybir as mybir
import concourse.tile as tile
from concourse import bacc
from concourse.bass_utils import run_bass_kernel_spmd

F32 = mybir.dt.float32
BF16 = mybir.dt.bfloat16
AF = mybir.ActivationFunctionType

EMBED_DIM = 1024
NUM_HEADS = 16
SEQ = 4096
BATCH = 1
N_CORES = 8


def _build_nc(S=SEQ, E=EMBED_DIM):
    EC = 128          # per-core feature slice (2 heads x 64)
    NI = E // 128     # contraction tiles for projections
    NQB = S // 512    # q blocks
    NKT = S // 128    # k tiles

    nc = bacc.Bacc(None, target_bir_lowering=False, debug=False)

    # x arrives pre-permuted to the SBUF layout: xP[p, sb, it, s'] =
    # x[sb*512+s', it*128+p] -- one contiguous 8KB line per partition per
    # 512-column s-block (full-rate DMA, no mid-dim segmentation)
    xP = nc.dram_tensor("xP", [128, S // 512, E // 128, 512], BF16,
                        kind="ExternalInput")
    # projection weights arrive pre-packed as [128, NI*EC]:
    # packed[p, it*EC + e] = W.T[it*128 + p, e]  (contiguous DMA lines)
    wqT = nc.dram_tensor("wqT", [128, NI * EC], BF16, kind="ExternalInput")
    wkT = nc.dram_tensor("wkT", [128, NI * EC], BF16, kind="ExternalInput")
    wvT = nc.dram_tensor("wvT", [128, NI * EC], BF16, kind="ExternalInput")
    woT = nc.dram_tensor("woT", [EC, E], BF16, kind="ExternalInput")
    bq = nc.dram_tensor("bq", [EC, 1], F32, kind="ExternalInput")
    bk = nc.dram_tensor("bk", [EC, 1], F32, kind="ExternalInput")
    bv = nc.dram_tensor("bv", [1, EC], F32, kind="ExternalInput")
    maskst = nc.dram_tensor("maskst", [128, 896], BF16, kind="ExternalInput")
    ident = nc.dram_tensor("ident", [128, 128], BF16, kind="ExternalInput")
    out = nc.dram_tensor("out", [S, E], BF16, kind="ExternalOutput")

    with tile.TileContext(nc) as tc:
        with tc.tile_pool(name="const", bufs=1) as const:
            w_sb = {}
            for name in ("q", "k", "v"):
                w_sb[name] = const.tile([128, NI, EC], BF16, tag=f"w{name}",
                                        name=f"w{name}")
            xt_sb = const.tile([128, S // 512, NI, 512], BF16, tag="xt")
            bq_sb = const.tile([128, 1], F32, tag="bq")
            bk_sb = const.tile([128, 1], F32, tag="bk")
            bv_row = const.tile([1, EC], F32, tag="bvr")
            bv_bc = const.tile([128, EC], F32, tag="bv")
            mask_sb = const.tile([128, 896], BF16, tag="mask")
            wo_sb = const.tile([128, E], BF16, tag="wo")
            id_sb = const.tile([128, 128], BF16, tag="ident")
            warm_src = const.tile([128, 260], BF16, tag="warmsrc")
            warm_act = const.tile([128, 1], BF16, tag="warmact")
            nc.vector.memset(warm_src[:, :], 1.0)
            # preload the Exp activation table off the critical path
            nc.scalar.activation(warm_act[:, :], warm_src[:, 0:1], AF.Exp)

            # DMA issue order = arrival order: q weights, first x half-chunk,
            # k weights, ... so the first projection can start ~4.5us in.
            # x streams in half-s-block chunks (4KB/partition contiguous).
            def xchunk(sb, h):
                nc.sync.dma_start(out=xt_sb[:, sb, 4 * h:4 * h + 4, :],
                                  in_=xP[:, sb, 4 * h:4 * h + 4, :])

            nc.sync.dma_start(
                out=w_sb["q"][:, :, :],
                in_=wqT.ap().rearrange("p (t e) -> p t e", t=NI))
            # first s-block in quarter chunks so projection it-tiles start
            # as soon as each 2-it slice lands
            for qtr in range(2):
                nc.sync.dma_start(out=xt_sb[:, 0, 2 * qtr:2 * qtr + 2, :],
                                  in_=xP[:, 0, 2 * qtr:2 * qtr + 2, :])
            nc.sync.dma_start(
                out=w_sb["k"][:, :, :],
                in_=wkT.ap().rearrange("p (t e) -> p t e", t=NI))
            for qtr in range(2, 4):
                nc.sync.dma_start(out=xt_sb[:, 0, 2 * qtr:2 * qtr + 2, :],
                                  in_=xP[:, 0, 2 * qtr:2 * qtr + 2, :])
            nc.sync.dma_start(out=bq_sb, in_=bq[:, :])
            nc.sync.dma_start(out=bk_sb, in_=bk[:, :])
            nc.sync.dma_start(out=bv_row, in_=bv[:, :])
            nc.sync.dma_start(out=mask_sb, in_=maskst[:, :])
            nc.sync.dma_start(out=id_sb, in_=ident[:, :])
            nc.sync.dma_start(
                out=w_sb["v"][:, :, :],
                in_=wvT.ap().rearrange("p (t e) -> p t e", t=NI))
            xchunk(1, 0)
            xchunk(1, 1)
            nc.sync.dma_start(out=wo_sb, in_=woT[:, :])
            for sb in range(2, S // 512):
                xchunk(sb, 0)
                xchunk(sb, 1)

            nc.gpsimd.partition_broadcast(bv_bc[:, :], bv_row[0:1, :])

            qt_sb = const.tile([128, S], BF16, tag="qt")
            kt_sb = const.tile([128, S], BF16, tag="kt")
            v_sb = const.tile([128, NKT, 130], BF16, tag="v")
            nc.vector.memset(v_sb[:, :, 64:65], 1.0)
            nc.vector.memset(v_sb[:, :, 129:130], 1.0)

            # PSUM banks: sc 2x2 + acc0/acc1 1x1 each + op 2x1 = 8
            with tc.tile_pool(name="ps", bufs=1, space="PSUM") as ps_pool, \
                 tc.tile_pool(name="spt", bufs=8) as spt, \
                 tc.tile_pool(name="satt", bufs=2) as satt, \
                 tc.tile_pool(name="satT", bufs=4) as satT, \
                 tc.tile_pool(name="srcp", bufs=4) as srcp, \
                 tc.tile_pool(name="sstage", bufs=4) as sstage:

                qk_emitted = [0]  # highest sb with q/k projection emitted
                qkproj_ps = {}

                def emit_qkproj_half(name, dst, bias, sb, half):
                    # half 0 emits its 0..3, half 1 its 4..7 + bias add, so
                    # score matmuls can interleave mid-projection and keep
                    # the exp stream fed
                    w = w_sb[name]
                    if half == 0:
                        qkproj_ps[(name, sb)] = ps_pool.tile(
                            [128, 512], F32, tag="op", bufs=2,
                            name=f"pj{name}{sb}")
                    ps = qkproj_ps[(name, sb)]
                    for it in range(4 * half, 4 * half + 4):
                        nc.tensor.matmul(
                            ps[:, 0:512],
                            lhsT=w[:, it, :],
                            rhs=xt_sb[:, sb, it, :],
                            start=(it == 0), stop=(it == NI - 1),
                        )
                    if half == 1:
                        nc.vector.tensor_scalar_add(
                            dst[:, sb * 512:(sb + 1) * 512], ps[:, 0:512],
                            bias[:, 0:1])
                        if name == "k":
                            qk_emitted[0] = max(qk_emitted[0], sb)

                def emit_qkproj_one(name, dst, bias, sb):
                    emit_qkproj_half(name, dst, bias, sb, 0)
                    emit_qkproj_half(name, dst, bias, sb, 1)

                wv = w_sb["v"]
                vproj_done = [0]

                def emit_vproj_one(st):
                    ps = ps_pool.tile([128, 512], F32, tag="op", bufs=2,
                                      name=f"pjv{st}")
                    for it in range(NI):
                        nc.tensor.matmul(
                            ps[:, 0:EC],
                            lhsT=xt_sb[:, st // 4, it,
                                       (st % 4) * 128:(st % 4) * 128 + 128],
                            rhs=wv[:, it, :],
                            start=(it == 0), stop=(it == NI - 1),
                        )
                    nc.vector.tensor_add(
                        v_sb[:, st, 0:64], ps[:, 0:64], bv_bc[:, 0:64])
                    nc.vector.tensor_add(
                        v_sb[:, st, 65:129], ps[:, 64:128], bv_bc[:, 64:128])

                # attn.V in transposed orientation: for each 128-wide q
                # subtile and head, acc_h[q, 0:65] += pT_h.T @ V'_h.
                # A start=True matmul zeroes the acc bank's WHOLE 2KB zero
                # region, so exactly one start (first matmul into the bank)
                # and one stop (last matmul, the qt=3 diagonal) per block --
                # the 4 packed q-subtile regions share the zeroing.
                def emit_attnv(acc, jpt, qb):
                    j, pt, off, r = jpt
                    nkt = 4 * (qb + 1)
                    for h in range(2):
                        for qt in range(max(r, 0), 4):
                            nc.tensor.matmul(
                                acc[h][:, qt * 65:qt * 65 + 65],
                                lhsT=pt[:, 512 * h + qt * 128 - off:
                                        512 * h + qt * 128 - off + 128],
                                rhs=v_sb[:, j, 65 * h:65 * h + 65],
                                start=(j == 0 and qt == max(r, 0)),
                                stop=(j == nkt - 1),
                                skip_group_check=True,
                            )

                # copy-engine rotation for PSUM drains (Pool-heavy; DVE help)
                drain_rr = [0]

                def drain_copy(dst, src, tail=False, qb=0):
                    # GPSIMD cannot read PSUM on hardware: drains live on DVE,
                    # with ScalarE helping while it still has exp slack
                    if tail:
                        engines = (nc.vector, nc.scalar)
                    else:
                        engines = (nc.vector,)
                    e = engines[drain_rr[0] % len(engines)]
                    drain_rr[0] += 1
                    if e is nc.scalar:
                        e.copy(dst, src)
                    else:
                        e.tensor_copy(dst, src)

                def emit_norm(qb, acc, att, rcp, qt=None, split=False):
                    # 1/denominator; qt=None does all 4 q-subtiles at once
                    qts = range(4) if qt is None else (qt,)
                    for h in range(2):
                        if qt is None:
                            a = acc[h][:, :]
                            den = bass.AP(tensor=a.tensor,
                                          offset=a.offset + 64,
                                          ap=[a.ap[0], [65, 4]])
                            nc.vector.reciprocal(rcp[:, 4 * h:4 * h + 4], den)
                        else:
                            nc.vector.reciprocal(
                                rcp[:, 4 * h + qt:4 * h + qt + 1],
                                acc[h][:, qt * 65 + 64:qt * 65 + 65])
                    for q in qts:
                        for h in range(2):
                            # split puts head 1 on ScalarE (exp-free in the
                            # endgame) so the tail transpose starts sooner
                            if split and h == 1:
                                nc.scalar.mul(
                                    att[:, q * 128 + 64 * h:
                                        q * 128 + 64 * h + 64],
                                    acc[h][:, q * 65:q * 65 + 64],
                                    rcp[:, 4 * h + q:4 * h + q + 1])
                            else:
                                nc.vector.tensor_scalar_mul(
                                    att[:, q * 128 + 64 * h:
                                        q * 128 + 64 * h + 64],
                                    acc[h][:, q * 65:q * 65 + 64],
                                    rcp[:, 4 * h + q:4 * h + q + 1])

                def emit_transpose(qb, att, attT, qt):
                    # PE transpose via identity: [128q, 128d] -> [128d, 128q]
                    trT = ps_pool.tile([128, 128], BF16, tag="op", bufs=2,
                                       name=f"tr{qb}_{qt}")
                    nc.tensor.transpose(trT[:, :],
                                        att[:, qt * 128:(qt + 1) * 128],
                                        id_sb[:, :])
                    nc.vector.tensor_copy(attT[:, qt * 128:(qt + 1) * 128],
                                          trT[:, :])

                def emit_oproj_one(qb, qt, nh, attT, stage, tail=False):
                    op = ps_pool.tile([128, 512], F32, tag="op", bufs=2,
                                      name=f"op{qb}_{qt}_{nh}")
                    nc.tensor.matmul(
                        op[:, :],
                        lhsT=attT[:, qt * 128:(qt + 1) * 128],
                        rhs=wo_sb[:, nh * 512:(nh + 1) * 512],
                        start=True, stop=True,
                    )
                    drain_copy(stage[:, qt, nh * 512:(nh + 1) * 512],
                               op[:, :], tail=tail, qb=qb)
                    if nh == 1:
                        nc.sync.dma_start(
                            out=out[qb * 512 + qt * 128:
                                    qb * 512 + (qt + 1) * 128, :],
                            in_=stage[:, qt, :])

                # global tile stream: (qb, j) in consumption order; the
                # scores->exp stage runs AHEAD tiles in front of the attn.V
                # stage so ScalarE saturates during the PE-heavy early blocks
                AHEAD = 28
                TILES = [(qb, j) for qb in range(NQB)
                         for j in range(4 * (qb + 1))]
                GIDX = {t: i for i, t in enumerate(TILES)}
                ptmap = {}
                cursor = [0]

                def emit_exp_tile(gi):
                    eqb, j = TILES[gi]
                    r = j - 4 * eqb  # >= 0 on the causal diagonal
                    off = 128 * r if r > 0 else 0
                    w = 512 - off   # valid q columns for this k-tile
                    sc = ps_pool.tile([128, 1024], F32, tag="sc", bufs=2,
                                      name=f"sc{eqb}_{j}")
                    for h in range(2):
                        hp = slice(64 * h, 64 * h + 64)
                        nc.tensor.matmul(
                            sc[:, 512 * h:512 * h + w],
                            lhsT=kt_sb[hp, j * 128:(j + 1) * 128],
                            rhs=qt_sb[hp, eqb * 512 + off:(eqb + 1) * 512],
                            start=True, stop=True,
                        )
                    if eqb == 1:
                        pt = spt.tile([128, 1024], BF16, tag="pt1", bufs=8,
                                      name=f"pt{eqb}_{j}")
                    else:
                        pt = spt.tile([128, 1024], BF16, tag="pt", bufs=26,
                                      name=f"pt{eqb}_{j}")
                    if r >= 0:
                        # one exp over both heads' [0:w] and [512:512+w]
                        # slices via a strided AP
                        def _two(t, w=w):
                            a = t[:, :]
                            return bass.AP(tensor=a.tensor, offset=a.offset,
                                           ap=[a.ap[0], [512, 2], [1, w]])
                        nc.scalar.activation(_two(pt), _two(sc), AF.Exp)
                        m = mask_sb[:, 384:384 + w]
                        for h in range(2):
                            pslc = pt[:, 512 * h:512 * h + w]
                            nc.vector.tensor_mul(pslc, pslc, m)
                    else:
                        nc.scalar.activation(pt[:, :], sc[:, :], AF.Exp)
                    ptmap[gi] = (j, pt, off, r)

                def advance_exp(upto):
                    while cursor[0] < min(upto, len(TILES)) and \
                            TILES[cursor[0]][0] <= qk_emitted[0]:
                        emit_exp_tile(cursor[0])
                        cursor[0] += 1

                # HAM warmup: cheap matmuls into the (not yet used) acc banks
                # while the first DMAs are in flight, so pe_busy_start lands
                # early and the real projections run at the warm clock.
                for i in range(12):
                    wp = ps_pool.tile([128, 260], F32, tag=f"acc{i % 2}",
                                      name=f"warm{i}")
                    nc.tensor.matmul(wp[:, :], lhsT=warm_src[:, 0:128],
                                     rhs=warm_src[:, :], start=True, stop=True)

                emit_qkproj_one("q", qt_sb, bq_sb, 0)
                emit_qkproj_one("k", kt_sb, bk_sb, 0)
                for st in range(4):
                    emit_vproj_one(st)
                vproj_done[0] = 4

                pending_epi = []   # prev-qb norm+transposes (must precede
                                   # this qb's first attn.V into acc)
                pending = []       # deferrable oproj items (1-2 qb backlog)

                qk_scheduled = [1]
                # block 1 is consumed last: its (early-computed) exps vacate
                # the ACT-bound endgame and its attn.V+epilogue give the tail
                # PE work while ScalarE drains
                BLOCK_ORDER = [0, 2, 3, 4, 5, 6, 7, 1]

                for pos, qb in enumerate(BLOCK_ORDER):
                    # bg items are CHAINS: multi-part chains keep their "op"
                    # psum tile across parts, so parts must be emitted with
                    # no other op-tag allocation in between
                    bg = []
                    for sb in range(qk_scheduled[0], min(qb + 3, NQB)):
                        for name, dst, bias in (("q", qt_sb, bq_sb),
                                                ("k", kt_sb, bk_sb)):
                            bg.append([
                                lambda n=name, d=dst, b=bias, s=sb, hf=hf:
                                emit_qkproj_half(n, d, b, s, hf)
                                for hf in range(2)])
                    qk_scheduled[0] = max(qk_scheduled[0], min(qb + 3, NQB))
                    nxt = BLOCK_ORDER[pos + 1] if pos + 1 < NQB else 0
                    lo = vproj_done[0]
                    hi = max(lo, 4 * (max(qb, nxt) + 1))
                    for st in range(lo, hi):
                        bg.append([lambda st=st: emit_vproj_one(st)])
                    vproj_done[0] = hi
                    chain = []

                    def pop_bg():
                        if not chain and bg:
                            chain.extend(bg.pop(0))
                        if chain:
                            chain.pop(0)()
                            return True
                        return False

                    nkt = 4 * (qb + 1)
                    last = pos == NQB - 1
                    tail_soon = pos == NQB - 2
                    acc = [ps_pool.tile([128, 260], F32, tag=f"acc{h}",
                                        name=f"acc{h}_{qb}")
                           for h in range(2)]
                    att = satt.tile([128, 512], BF16, tag="att",
                                    name=f"att{qb}")
                    attT = satT.tile([128, 512], BF16, tag="attT",
                                     name=f"attT{qb}")
                    rcp = srcp.tile([128, 8], F32, tag="rcp", name=f"rcp{qb}")
                    stage = sstage.tile([128, 4, E], BF16, tag="stage",
                                        name=f"stage{qb}")
                    reserve = 4 if last else (6 if qb >= 5 else 8)
                    for j in range(nkt):
                        gi = GIDX[(qb, j)]
                        advance_exp(gi + AHEAD)
                        if chain:
                            chain.pop(0)()            # finish open bg chain
                        elif j == 0 and pending_epi:
                            pending_epi.pop(0)()      # prev norm+transposes
                        elif j % 2 == 1 and bg:
                            pop_bg()                  # time-critical projs
                        elif len(pending) > reserve:
                            pending.pop(0)()          # prev oproj, one tile
                        else:
                            pop_bg()
                        if j == 3 and nkt <= 8:
                            pop_bg()                  # small blocks: drain bg
                        advance_exp(gi + AHEAD)
                        emit_attnv(acc, ptmap.pop(gi), qb)
                        if last and j >= 4 * qb:
                            # tail: per-q-subtile chains pipelined across
                            # engines right after the diagonal lands; spend
                            # the reserved oproj items in the norm latency
                            qt = j - 4 * qb
                            emit_norm(qb, acc, att, rcp, qt=qt,
                                      split=(qt >= 2))
                            if pending:
                                pending.pop(0)()
                            emit_transpose(qb, att, attT, qt)
                            for nh in range(2):
                                emit_oproj_one(qb, qt, nh, attT, stage,
                                               tail=True)
                    while chain or bg:
                        pop_bg()

                    if not last:
                        # cap the oproj backlog at one block so tile-pool
                        # buffer reuse can't order a writer before its reader
                        while len(pending) > 24:
                            pending.pop(0)()

                        def epi(qb=qb, acc=acc, att=att, attT=attT,
                                rcp=rcp, sp=tail_soon):
                            emit_norm(qb, acc, att, rcp, split=sp)
                            for qt in range(4):
                                emit_transpose(qb, att, attT, qt)
                        pending_epi.append(epi)
                        for qt in range(4):
                            for nh in range(2):
                                pending.append(
                                    lambda qb=qb, qt=qt, nh=nh, a=attT,
                                    s=stage, tl=tail_soon:
                                    emit_oproj_one(qb, qt, nh, a, s, tail=tl))

    nc.compile()
    return nc


def _make_mask_strip():
    k = np.arange(128)[:, None]
    t = np.arange(896)[None, :]
    return (k <= t - 384).astype(np.float32)


def _pack_w(wT):
    # [E, EC] -> [128, NI*EC] with packed[p, it*EC+e] = wT[it*128+p, e]
    E, EC = wT.shape
    return np.ascontiguousarray(
        wT.reshape(E // 128, 128, EC).transpose(1, 0, 2).reshape(128, -1))


def _shard_inputs(x, Wq, bq, Wk, bk, Wv, bv, Wo):
    import ml_dtypes
    bf16 = ml_dtypes.bfloat16
    S, E = x.shape[-2], x.shape[-1]
    xP = np.ascontiguousarray(
        np.asarray(x, np.float32).reshape(S // 512, 512, E // 128, 128)
        .transpose(3, 0, 2, 1)).astype(bf16)
    strip = _make_mask_strip().astype(bf16)
    eye = np.eye(128, dtype=np.float32).astype(bf16)
    in_maps = []
    for c in range(N_CORES):
        sl = slice(128 * c, 128 * (c + 1))
        in_maps.append({
            "xP": xP,
            "wqT": _pack_w((np.asarray(Wq, np.float32)[sl, :] / 8.0).T).astype(bf16),
            "wkT": _pack_w(np.asarray(Wk, np.float32)[sl, :].T).astype(bf16),
            "wvT": _pack_w(np.asarray(Wv, np.float32)[sl, :].T).astype(bf16),
            "woT": np.ascontiguousarray(np.asarray(Wo, np.float32)[:, sl].T).astype(bf16),
            "bq": (np.asarray(bq, np.float32)[sl] / 8.0).reshape(128, 1),
            "bk": np.asarray(bk, np.float32)[sl].reshape(128, 1),
            "bv": np.asarray(bv, np.float32)[sl].reshape(1, 128),
            "maskst": strip,
            "ident": eye,
        })
    return in_maps


_NC_CACHE = {}


def kernel(x, Wq, bq, Wk, bk, Wv, bv, Wo, bo):
    x = np.asarray(x)
    B, S, E = x.shape
    if (S, E) not in _NC_CACHE:
        _NC_CACHE[(S, E)] = _build_nc(S=S, E=E)
    nc = _NC_CACHE[(S, E)]

    in_maps = _shard_inputs(x, Wq, bq, Wk, bk, Wv, bv, Wo)
    res = run_bass_kernel_spmd(nc, in_maps, list(range(N_CORES)))

    total = np.zeros((S, E), np.float32)
    for r in res.results:
        total += np.asarray(r["out"], np.float32).reshape(S, E)
    total += np.asarray(bo, np.float32)
    return total.reshape(B, S, E).astype(np.float32)


# revision 96
# speedup vs baseline: 1.1224x; 1.0053x over previous
"""Causal multi-head attention (B=1, S=4096, E=1024, H=16, Dk=64) on 8 TRN2
NeuronCores via Bass/Tile, head-sharded (tensor parallel): core c computes
heads 2c and 2c+1 end-to-end plus its partial output projection; the host sums
the 8 partials (bf16) and adds the output bias.

Per-core program (transposed attn.V + global exp-ahead pipeline):
  QT/KT[e'=128, S] = (W x^T + b) in bf16 (softmax 1/sqrt(Dk) folded into Wq/bq)
  V'[k, 2*65]      = x Wv^T + bv, with a ones column per head
  global tile stream, scores->exp running AHEAD of attn.V consumption:
    scoresT[k, q] via PE (2 heads packed with row tiling, d=64 each)
    pT = exp(scoresT) on ScalarE (no max subtraction; scores are ~N(0,1))
    diagonal tiles: multiply by causal 0/1 mask strip (post-exp)
    per q-subtile (128) and head: accT_h[q, 0:65] += pT_h.T @ V'_h
      (out partitions = 128 q values -> half the PE cycles of the
       [65, q] orientation; column 64 accumulates the softmax denom)
  att[q, d] = accT[q, 0:64] * (1/accT[q, 64])   (per-partition scalar on DVE)
  attT[d, q] via PE transpose (identity matmul), then
  partial[q, e] = attT.T @ Wo_c.T ; drained to bf16 partial output
"""

import numpy as np

import concourse.bass as bass
import concourse.mybir as mybir
import concourse.tile as tile
from concourse import bacc
from concourse.bass_utils import run_bass_kernel_spmd

F32 = mybir.dt.float32
BF16 = mybir.dt.bfloat16
AF = mybir.ActivationFunctionType

EMBED_DIM = 1024
NUM_HEADS = 16
SEQ = 4096
BATCH = 1
N_CORES = 8


def _build_nc(S=SEQ, E=EMBED_DIM):
    EC = 128          # per-core feature slice (2 heads x 64)
    NI = E // 128     # contraction tiles for projections
    NQB = S // 512    # q blocks
    NKT = S // 128    # k tiles

    nc = bacc.Bacc(None, target_bir_lowering=False, debug=False)

    # x arrives pre-permuted to the SBUF layout: xP[p, sb, it, s'] =
    # x[sb*512+s', it*128+p] -- one contiguous 8KB line per partition per
    # 512-column s-block (full-rate DMA, no mid-dim segmentation)
    xP = nc.dram_tensor("xP", [128, S // 512, E // 128, 512], BF16,
                        kind="ExternalInput")
    # projection weights arrive pre-packed as [128, NI*EC]:
    # packed[p, it*EC + e] = W.T[it*128 + p, e]  (contiguous DMA lines)
    wqT = nc.dram_tensor("wqT", [128, NI * EC], BF16, kind="ExternalInput")
    wkT = nc.dram_tensor("wkT", [128, NI * EC], BF16, kind="ExternalInput")
    wvT = nc.dram_tensor("wvT", [128, NI * EC], BF16, kind="ExternalInput")
    woT = nc.dram_tensor("woT", [EC, E], BF16, kind="ExternalInput")
    bq = nc.dram_tensor("bq", [EC, 1], F32, kind="ExternalInput")
    bk = nc.dram_tensor("bk", [EC, 1], F32, kind="ExternalInput")
    bv = nc.dram_tensor("bv", [1, EC], F32, kind="ExternalInput")
    maskst = nc.dram_tensor("maskst", [128, 896], BF16, kind="ExternalInput")
    ident = nc.dram_tensor("ident", [128, 128], BF16, kind="ExternalInput")
    out = nc.dram_tensor("out", [S, E], BF16, kind="ExternalOutput")

    with tile.TileContext(nc) as tc:
        with tc.tile_pool(name="const", bufs=1) as const:
            w_sb = {}
            for name in ("q", "k", "v"):
                w_sb[name] = const.tile([128, NI, EC], BF16, tag=f"w{name}",
                                        name=f"w{name}")
            xt_sb = const.tile([128, S // 512, NI, 512], BF16, tag="xt")
            bq_sb = const.tile([128, 1], F32, tag="bq")
            bk_sb = const.tile([128, 1], F32, tag="bk")
            bv_row = const.tile([1, EC], F32, tag="bvr")
            bv_bc = const.tile([128, EC], F32, tag="bv")
            mask_sb = const.tile([128, 896], BF16, tag="mask")
            wo_sb = const.tile([128, E], BF16, tag="wo")
            id_sb = const.tile([128, 128], BF16, tag="ident")
            warm_src = const.tile([128, 260], BF16, tag="warmsrc")
            warm_act = const.tile([128, 1], BF16, tag="warmact")
            nc.vector.memset(warm_src[:, :], 1.0)
            # preload the Exp activation table off the critical path
            nc.scalar.activation(warm_act[:, :], warm_src[:, 0:1], AF.Exp)

            # DMA issue order = arrival order: q weights, first x half-chunk,
            # k weights, ... so the first projection can start ~4.5us in.
            # x streams in half-s-block chunks (4KB/partition contiguous).
            def xchunk(sb, h):
                nc.sync.dma_start(out=xt_sb[:, sb, 4 * h:4 * h + 4, :],
                                  in_=xP[:, sb, 4 * h:4 * h + 4, :])

            nc.sync.dma_start(
                out=w_sb["q"][:, :, :],
                in_=wqT.ap().rearrange("p (t e) -> p t e", t=NI))
            # first s-block in quarter chunks so projection it-tiles start
            # as soon as each 2-it slice lands
            for qtr in range(2):
                nc.sync.dma_start(out=xt_sb[:, 0, 2 * qtr:2 * qtr + 2, :],
                                  in_=xP[:, 0, 2 * qtr:2 * qtr + 2, :])
            nc.sync.dma_start(
                out=w_sb["k"][:, :, :],
                in_=wkT.ap().rearrange("p (t e) -> p t e", t=NI))
            for qtr in range(2, 4):
                nc.sync.dma_start(out=xt_sb[:, 0, 2 * qtr:2 * qtr + 2, :],
                                  in_=xP[:, 0, 2 * qtr:2 * qtr + 2, :])
            nc.sync.dma_start(out=bq_sb, in_=bq[:, :])
            nc.sync.dma_start(out=bk_sb, in_=bk[:, :])
            nc.sync.dma_start(out=bv_row, in_=bv[:, :])
            nc.sync.dma_start(out=mask_sb, in_=maskst[:, :])
            nc.sync.dma_start(out=id_sb, in_=ident[:, :])
            nc.sync.dma_start(
                out=w_sb["v"][:, :, :],
                in_=wvT.ap().rearrange("p (t e) -> p t e", t=NI))
            xchunk(1, 0)
            xchunk(1, 1)
            nc.sync.dma_start(out=wo_sb, in_=woT[:, :])
            for sb in range(2, S // 512):
                xchunk(sb, 0)
                xchunk(sb, 1)

            nc.gpsimd.partition_broadcast(bv_bc[:, :], bv_row[0:1, :])

            qt_sb = const.tile([128, S], BF16, tag="qt")
            kt_sb = const.tile([128, S], BF16, tag="kt")
            v_sb = const.tile([128, NKT, 130], BF16, tag="v")
            nc.vector.memset(v_sb[:, :, 64:65], 1.0)
            nc.vector.memset(v_sb[:, :, 129:130], 1.0)

            # PSUM banks: sc 2x2 + acc0/acc1 1x1 each + op 2x1 = 8
            with tc.tile_pool(name="ps", bufs=1, space="PSUM") as ps_pool, \
                 tc.tile_pool(name="spt", bufs=8) as spt, \
                 tc.tile_pool(name="satt", bufs=2) as satt, \
                 tc.tile_pool(name="satT", bufs=4) as satT, \
                 tc.tile_pool(name="srcp", bufs=4) as srcp, \
                 tc.tile_pool(name="sstage", bufs=4) as sstage:

                qk_emitted = [0]  # highest sb with q/k projection emitted
                qkproj_ps = {}

                def emit_qkproj_half(name, dst, bias, sb, half):
                    # half 0 emits its 0..3, half 1 its 4..7 + bias add, so
                    # score matmuls can interleave mid-projection and keep
                    # the exp stream fed
                    w = w_sb[name]
                    if half == 0:
                        qkproj_ps[(name, sb)] = ps_pool.tile(
                            [128, 512], F32, tag="op", bufs=2,
                            name=f"pj{name}{sb}")
                    ps = qkproj_ps[(name, sb)]
                    for it in range(4 * half, 4 * half + 4):
                        nc.tensor.matmul(
                            ps[:, 0:512],
                            lhsT=w[:, it, :],
                            rhs=xt_sb[:, sb, it, :],
                            start=(it == 0), stop=(it == NI - 1),
                        )
                    if half == 1:
                        nc.vector.tensor_scalar_add(
                            dst[:, sb * 512:(sb + 1) * 512], ps[:, 0:512],
                            bias[:, 0:1])
                        if name == "k":
                            qk_emitted[0] = max(qk_emitted[0], sb)

                def emit_qkproj_one(name, dst, bias, sb):
                    emit_qkproj_half(name, dst, bias, sb, 0)
                    emit_qkproj_half(name, dst, bias, sb, 1)

                wv = w_sb["v"]
                vproj_done = [0]

                def emit_vproj_one(st):
                    ps = ps_pool.tile([128, 512], F32, tag="op", bufs=2,
                                      name=f"pjv{st}")
                    for it in range(NI):
                        nc.tensor.matmul(
                            ps[:, 0:EC],
                            lhsT=xt_sb[:, st // 4, it,
                                       (st % 4) * 128:(st % 4) * 128 + 128],
                            rhs=wv[:, it, :],
                            start=(it == 0), stop=(it == NI - 1),
                        )
                    nc.vector.tensor_add(
                        v_sb[:, st, 0:64], ps[:, 0:64], bv_bc[:, 0:64])
                    nc.vector.tensor_add(
                        v_sb[:, st, 65:129], ps[:, 64:128], bv_bc[:, 64:128])

                # attn.V in transposed orientation: for each 128-wide q
                # subtile and head, acc_h[q, 0:65] += pT_h.T @ V'_h.
                # A start=True matmul zeroes the acc bank's WHOLE 2KB zero
                # region, so exactly one start (first matmul into the bank)
                # and one stop (last matmul, the qt=3 diagonal) per block --
                # the 4 packed q-subtile regions share the zeroing.
                def emit_attnv(acc, jpt, qb):
                    j, pt, off, r = jpt
                    nkt = 4 * (qb + 1)
                    for h in range(2):
                        for qt in range(max(r, 0), 4):
                            nc.tensor.matmul(
                                acc[h][:, qt * 65:qt * 65 + 65],
                                lhsT=pt[:, 512 * h + qt * 128 - off:
                                        512 * h + qt * 128 - off + 128],
                                rhs=v_sb[:, j, 65 * h:65 * h + 65],
                                start=(j == 0 and qt == max(r, 0)),
                                stop=(j == nkt - 1),
                                skip_group_check=True,
                            )

                # copy-engine rotation for PSUM drains (Pool-heavy; DVE help)
                drain_rr = [0]

                def drain_copy(dst, src, tail=False, qb=0):
                    # GPSIMD cannot read PSUM on hardware: drains live on DVE,
                    # with ScalarE helping while it still has exp slack
                    if tail:
                        engines = (nc.vector, nc.scalar)
                    else:
                        engines = (nc.vector,)
                    e = engines[drain_rr[0] % len(engines)]
                    drain_rr[0] += 1
                    if e is nc.scalar:
                        e.copy(dst, src)
                    else:
                        e.tensor_copy(dst, src)

                def emit_norm(qb, acc, att, rcp, qt=None, split=False):
                    # 1/denominator; qt=None does all 4 q-subtiles at once
                    qts = range(4) if qt is None else (qt,)
                    for h in range(2):
                        if qt is None:
                            a = acc[h][:, :]
                            den = bass.AP(tensor=a.tensor,
                                          offset=a.offset + 64,
                                          ap=[a.ap[0], [65, 4]])
                            nc.vector.reciprocal(rcp[:, 4 * h:4 * h + 4], den)
                        else:
                            nc.vector.reciprocal(
                                rcp[:, 4 * h + qt:4 * h + qt + 1],
                                acc[h][:, qt * 65 + 64:qt * 65 + 65])
                    for q in qts:
                        for h in range(2):
                            # split puts head 1 on ScalarE (exp-free in the
                            # endgame) so the tail transpose starts sooner
                            if split and h == 1:
                                nc.scalar.mul(
                                    att[:, q * 128 + 64 * h:
                                        q * 128 + 64 * h + 64],
                                    acc[h][:, q * 65:q * 65 + 64],
                                    rcp[:, 4 * h + q:4 * h + q + 1])
                            else:
                                nc.vector.tensor_scalar_mul(
                                    att[:, q * 128 + 64 * h:
                                        q * 128 + 64 * h + 64],
                                    acc[h][:, q * 65:q * 65 + 64],
                                    rcp[:, 4 * h + q:4 * h + q + 1])

                def emit_transpose(qb, att, attT, qt):
                    # PE transpose via identity: [128q, 128d] -> [128d, 128q]
                    trT = ps_pool.tile([128, 128], BF16, tag="op", bufs=2,
                                       name=f"tr{qb}_{qt}")
                    nc.tensor.transpose(trT[:, :],
                                        att[:, qt * 128:(qt + 1) * 128],
                                        id_sb[:, :])
                    nc.vector.tensor_copy(attT[:, qt * 128:(qt + 1) * 128],
                                          trT[:, :])

                def emit_oproj_one(qb, qt, nh, attT, stage, tail=False):
                    op = ps_pool.tile([128, 512], F32, tag="op", bufs=2,
                                      name=f"op{qb}_{qt}_{nh}")
                    nc.tensor.matmul(
                        op[:, :],
                        lhsT=attT[:, qt * 128:(qt + 1) * 128],
                        rhs=wo_sb[:, nh * 512:(nh + 1) * 512],
                        start=True, stop=True,
                    )
                    drain_copy(stage[:, qt, nh * 512:(nh + 1) * 512],
                               op[:, :], tail=tail, qb=qb)
                    if nh == 1:
                        nc.sync.dma_start(
                            out=out[qb * 512 + qt * 128:
                                    qb * 512 + (qt + 1) * 128, :],
                            in_=stage[:, qt, :])

                # global tile stream: (qb, j) in consumption order; the
                # scores->exp stage runs AHEAD tiles in front of the attn.V
                # stage so ScalarE saturates during the PE-heavy early blocks
                AHEAD = 32
                TILES = [(qb, j) for qb in range(NQB)
                         for j in range(4 * (qb + 1))]
                GIDX = {t: i for i, t in enumerate(TILES)}
                ptmap = {}
                cursor = [0]

                def emit_exp_tile(gi):
                    eqb, j = TILES[gi]
                    r = j - 4 * eqb  # >= 0 on the causal diagonal
                    off = 128 * r if r > 0 else 0
                    w = 512 - off   # valid q columns for this k-tile
                    sc = ps_pool.tile([128, 1024], F32, tag="sc", bufs=2,
                                      name=f"sc{eqb}_{j}")
                    for h in range(2):
                        hp = slice(64 * h, 64 * h + 64)
                        nc.tensor.matmul(
                            sc[:, 512 * h:512 * h + w],
                            lhsT=kt_sb[hp, j * 128:(j + 1) * 128],
                            rhs=qt_sb[hp, eqb * 512 + off:(eqb + 1) * 512],
                            start=True, stop=True,
                        )
                    if eqb == 1:
                        pt = spt.tile([128, 1024], BF16, tag="pt1", bufs=8,
                                      name=f"pt{eqb}_{j}")
                    else:
                        pt = spt.tile([128, 1024], BF16, tag="pt", bufs=26,
                                      name=f"pt{eqb}_{j}")
                    if r >= 0:
                        # one exp over both heads' [0:w] and [512:512+w]
                        # slices via a strided AP
                        def _two(t, w=w):
                            a = t[:, :]
                            return bass.AP(tensor=a.tensor, offset=a.offset,
                                           ap=[a.ap[0], [512, 2], [1, w]])
                        nc.scalar.activation(_two(pt), _two(sc), AF.Exp)
                        m = mask_sb[:, 384:384 + w]
                        for h in range(2):
                            pslc = pt[:, 512 * h:512 * h + w]
                            nc.vector.tensor_mul(pslc, pslc, m)
                    else:
                        nc.scalar.activation(pt[:, :], sc[:, :], AF.Exp)
                    ptmap[gi] = (j, pt, off, r)

                def advance_exp(upto):
                    while cursor[0] < min(upto, len(TILES)) and \
                            TILES[cursor[0]][0] <= qk_emitted[0]:
                        emit_exp_tile(cursor[0])
                        cursor[0] += 1

                # HAM warmup: cheap matmuls into the (not yet used) acc banks
                # while the first DMAs are in flight, so pe_busy_start lands
                # early and the real projections run at the warm clock.
                for i in range(12):
                    wp = ps_pool.tile([128, 260], F32, tag=f"acc{i % 2}",
                                      name=f"warm{i}")
                    nc.tensor.matmul(wp[:, :], lhsT=warm_src[:, 0:128],
                                     rhs=warm_src[:, :], start=True, stop=True)

                emit_qkproj_one("q", qt_sb, bq_sb, 0)
                emit_qkproj_one("k", kt_sb, bk_sb, 0)
                for st in range(4):
                    emit_vproj_one(st)
                vproj_done[0] = 4

                pending_epi = []   # prev-qb norm+transposes (must precede
                                   # this qb's first attn.V into acc)
                pending = []       # deferrable oproj items (1-2 qb backlog)

                qk_scheduled = [1]
                # block 1 is consumed last: its (early-computed) exps vacate
                # the ACT-bound endgame and its attn.V+epilogue give the tail
                # PE work while ScalarE drains
                BLOCK_ORDER = [0, 2, 3, 4, 5, 6, 7, 1]

                for pos, qb in enumerate(BLOCK_ORDER):
                    # bg items are CHAINS: multi-part chains keep their "op"
                    # psum tile across parts, so parts must be emitted with
                    # no other op-tag allocation in between
                    bg = []
                    for sb in range(qk_scheduled[0], min(qb + 3, NQB)):
                        for name, dst, bias in (("q", qt_sb, bq_sb),
                                                ("k", kt_sb, bk_sb)):
                            bg.append([
                                lambda n=name, d=dst, b=bias, s=sb, hf=hf:
                                emit_qkproj_half(n, d, b, s, hf)
                                for hf in range(2)])
                    qk_scheduled[0] = max(qk_scheduled[0], min(qb + 3, NQB))
                    nxt = BLOCK_ORDER[pos + 1] if pos + 1 < NQB else 0
                    lo = vproj_done[0]
                    hi = max(lo, 4 * (max(qb, nxt) + 1))
                    for st in range(lo, hi):
                        bg.append([lambda st=st: emit_vproj_one(st)])
                    vproj_done[0] = hi
                    chain = []

                    def pop_bg():
                        if not chain and bg:
                            chain.extend(bg.pop(0))
                        if chain:
                            chain.pop(0)()
                            return True
                        return False

                    nkt = 4 * (qb + 1)
                    last = pos == NQB - 1
                    tail_soon = pos == NQB - 2
                    acc = [ps_pool.tile([128, 260], F32, tag=f"acc{h}",
                                        name=f"acc{h}_{qb}")
                           for h in range(2)]
                    att = satt.tile([128, 512], BF16, tag="att",
                                    name=f"att{qb}")
                    attT = satT.tile([128, 512], BF16, tag="attT",
                                     name=f"attT{qb}")
                    rcp = srcp.tile([128, 8], F32, tag="rcp", name=f"rcp{qb}")
                    stage = sstage.tile([128, 4, E], BF16, tag="stage",
                                        name=f"stage{qb}")
                    reserve = 4 if last else (5 if qb >= 5 else 8)
                    for j in range(nkt):
                        gi = GIDX[(qb, j)]
                        advance_exp(gi + AHEAD)
                        if chain:
                            chain.pop(0)()            # finish open bg chain
                        elif j == 0 and pending_epi:
                            pending_epi.pop(0)()      # prev norm+transposes
                        elif j % 2 == 1 and bg:
                            pop_bg()                  # time-critical projs
                        elif len(pending) > reserve:
                            pending.pop(0)()          # prev oproj, one tile
                        else:
                            pop_bg()
                        if j == 3 and nkt <= 8:
                            pop_bg()                  # small blocks: drain bg
                        advance_exp(gi + AHEAD)
                        emit_attnv(acc, ptmap.pop(gi), qb)
                        if last and j >= 4 * qb:
                            # tail: per-q-subtile chains pipelined across
                            # engines right after the diagonal lands; spend
                            # the reserved oproj items in the norm latency
                            qt = j - 4 * qb
                            emit_norm(qb, acc, att, rcp, qt=qt,
                                      split=(qt >= 2))
                            if pending:
                                pending.pop(0)()
                            emit_transpose(qb, att, attT, qt)
                            for nh in range(2):
                                emit_oproj_one(qb, qt, nh, attT, stage,
                                               tail=True)
                    while chain or bg:
                        pop_bg()

                    if not last:
                        # cap the oproj backlog at one block so tile-pool
                        # buffer reuse can't order a writer before its reader
                        while len(pending) > 24:
                            pending.pop(0)()

                        def epi(qb=qb, acc=acc, att=att, attT=attT,
                                rcp=rcp, sp=tail_soon):
                            emit_norm(qb, acc, att, rcp, split=sp)
                            for qt in range(4):
                                emit_transpose(qb, att, attT, qt)
                        pending_epi.append(epi)
                        for qt in range(4):
                            for nh in range(2):
                                pending.append(
                                    lambda qb=qb, qt=qt, nh=nh, a=attT,
                                    s=stage, tl=tail_soon:
                                    emit_oproj_one(qb, qt, nh, a, s, tail=tl))

    nc.compile()
    return nc


def _make_mask_strip():
    k = np.arange(128)[:, None]
    t = np.arange(896)[None, :]
    return (k <= t - 384).astype(np.float32)


def _pack_w(wT):
    # [E, EC] -> [128, NI*EC] with packed[p, it*EC+e] = wT[it*128+p, e]
    E, EC = wT.shape
    return np.ascontiguousarray(
        wT.reshape(E // 128, 128, EC).transpose(1, 0, 2).reshape(128, -1))


def _shard_inputs(x, Wq, bq, Wk, bk, Wv, bv, Wo):
    import ml_dtypes
    bf16 = ml_dtypes.bfloat16
    S, E = x.shape[-2], x.shape[-1]
    xP = np.ascontiguousarray(
        np.asarray(x, np.float32).reshape(S // 512, 512, E // 128, 128)
        .transpose(3, 0, 2, 1)).astype(bf16)
    strip = _make_mask_strip().astype(bf16)
    eye = np.eye(128, dtype=np.float32).astype(bf16)
    in_maps = []
    for c in range(N_CORES):
        sl = slice(128 * c, 128 * (c + 1))
        in_maps.append({
            "xP": xP,
            "wqT": _pack_w((np.asarray(Wq, np.float32)[sl, :] / 8.0).T).astype(bf16),
            "wkT": _pack_w(np.asarray(Wk, np.float32)[sl, :].T).astype(bf16),
            "wvT": _pack_w(np.asarray(Wv, np.float32)[sl, :].T).astype(bf16),
            "woT": np.ascontiguousarray(np.asarray(Wo, np.float32)[:, sl].T).astype(bf16),
            "bq": (np.asarray(bq, np.float32)[sl] / 8.0).reshape(128, 1),
            "bk": np.asarray(bk, np.float32)[sl].reshape(128, 1),
            "bv": np.asarray(bv, np.float32)[sl].reshape(1, 128),
            "maskst": strip,
            "ident": eye,
        })
    return in_maps


_NC_CACHE = {}


def kernel(x, Wq, bq, Wk, bk, Wv, bv, Wo, bo):
    x = np.asarray(x)
    B, S, E = x.shape
    if (S, E) not in _NC_CACHE:
        _NC_CACHE[(S, E)] = _build_nc(S=S, E=E)
    nc = _NC_CACHE[(S, E)]

    in_maps = _shard_inputs(x, Wq, bq, Wk, bk, Wv, bv, Wo)
    res = run_bass_kernel_spmd(nc, in_maps, list(range(N_CORES)))

    total = np.zeros((S, E), np.float32)
    for r in res.results:
        total += np.asarray(r["out"], np.float32).reshape(S, E)
    total += np.asarray(bo, np.float32)
    return total.reshape(B, S, E).astype(np.float32)


# revision 97
# speedup vs baseline: 1.1282x; 1.0052x over previous
"""Causal multi-head attention (B=1, S=4096, E=1024, H=16, Dk=64) on 8 TRN2
NeuronCores via Bass/Tile, head-sharded (tensor parallel): core c computes
heads 2c and 2c+1 end-to-end plus its partial output projection; the host sums
the 8 partials (bf16) and adds the output bias.

Per-core program (transposed attn.V + global exp-ahead pipeline):
  QT/KT[e'=128, S] = (W x^T + b) in bf16 (softmax 1/sqrt(Dk) folded into Wq/bq)
  V'[k, 2*65]      = x Wv^T + bv, with a ones column per head
  global tile stream, scores->exp running AHEAD tiles in front of attn.V
  consumption so ScalarE (the co-bottleneck at ~141us busy, vs ~143us on PE)
  stays saturated through the PE-heavy projection phase:
    scoresT[k, q] via PE (2 heads packed with row tiling, d=64 each)
    pT = exp(scoresT) on ScalarE (no max subtraction; scores are ~N(0,1))
    diagonal tiles: multiply by causal 0/1 mask strip (post-exp)
    per q-subtile (128) and head: accT_h[q, 0:65] += pT_h.T @ V'_h
      (out partitions = 128 q values -> half the PE cycles of the
       [65, q] orientation; column 64 accumulates the softmax denom;
       the 4 q-subtile regions share one PSUM bank, so exactly one
       start/stop pair per bank per block -- start zeroes the whole
       2KB zero region)
  att[q, d] = accT[q, 0:64] * (1/accT[q, 64])   (per-partition scalar on DVE)
  attT[d, q] via PE transpose (identity matmul), then
  partial[q, e] = attT.T @ Wo_c.T ; drained to bf16 partial output.
  Output-projection work is deferred (a standing ~8-16 item backlog) into
  the exp-bound late blocks to fill PE; PSUM drains live on DVE only
  (GPSIMD cannot read PSUM), with ScalarE helping once exps are done.
"""

import numpy as np

import concourse.bass as bass
import concourse.mybir as mybir
import concourse.tile as tile
from concourse import bacc
from concourse.bass_utils import run_bass_kernel_spmd

F32 = mybir.dt.float32
BF16 = mybir.dt.bfloat16
AF = mybir.ActivationFunctionType

EMBED_DIM = 1024
NUM_HEADS = 16
SEQ = 4096
BATCH = 1
N_CORES = 8


def _build_nc(S=SEQ, E=EMBED_DIM):
    EC = 128          # per-core feature slice (2 heads x 64)
    NI = E // 128     # contraction tiles for projections
    NQB = S // 512    # q blocks
    NKT = S // 128    # k tiles

    nc = bacc.Bacc(None, target_bir_lowering=False, debug=False)

    # x arrives pre-permuted to the SBUF layout: xP[p, sb, it, s'] =
    # x[sb*512+s', it*128+p] -- one contiguous 8KB line per partition per
    # 512-column s-block (full-rate DMA, no mid-dim segmentation)
    xP = nc.dram_tensor("xP", [128, S // 512, E // 128, 512], BF16,
                        kind="ExternalInput")
    # projection weights arrive pre-packed as [128, NI*EC]:
    # packed[p, it*EC + e] = W.T[it*128 + p, e]  (contiguous DMA lines)
    wqT = nc.dram_tensor("wqT", [128, NI * EC], BF16, kind="ExternalInput")
    wkT = nc.dram_tensor("wkT", [128, NI * EC], BF16, kind="ExternalInput")
    wvT = nc.dram_tensor("wvT", [128, NI * EC], BF16, kind="ExternalInput")
    woT = nc.dram_tensor("woT", [EC, E], BF16, kind="ExternalInput")
    bq = nc.dram_tensor("bq", [EC, 1], F32, kind="ExternalInput")
    bk = nc.dram_tensor("bk", [EC, 1], F32, kind="ExternalInput")
    bv = nc.dram_tensor("bv", [1, EC], F32, kind="ExternalInput")
    maskst = nc.dram_tensor("maskst", [128, 896], BF16, kind="ExternalInput")
    ident = nc.dram_tensor("ident", [128, 128], BF16, kind="ExternalInput")
    out = nc.dram_tensor("out", [S, E], BF16, kind="ExternalOutput")

    with tile.TileContext(nc) as tc:
        with tc.tile_pool(name="const", bufs=1) as const:
            w_sb = {}
            for name in ("q", "k", "v"):
                w_sb[name] = const.tile([128, NI, EC], BF16, tag=f"w{name}",
                                        name=f"w{name}")
            xt_sb = const.tile([128, S // 512, NI, 512], BF16, tag="xt")
            bq_sb = const.tile([128, 1], F32, tag="bq")
            bk_sb = const.tile([128, 1], F32, tag="bk")
            bv_row = const.tile([1, EC], F32, tag="bvr")
            bv_bc = const.tile([128, EC], F32, tag="bv")
            mask_sb = const.tile([128, 896], BF16, tag="mask")
            wo_sb = const.tile([128, E], BF16, tag="wo")
            id_sb = const.tile([128, 128], BF16, tag="ident")
            warm_src = const.tile([128, 260], BF16, tag="warmsrc")
            warm_act = const.tile([128, 1], BF16, tag="warmact")
            nc.vector.memset(warm_src[:, :], 1.0)
            # preload the Exp activation table off the critical path
            nc.scalar.activation(warm_act[:, :], warm_src[:, 0:1], AF.Exp)

            # DMA issue order = arrival order: q weights, first x half-chunk,
            # k weights, ... so the first projection can start ~4.5us in.
            # x streams in half-s-block chunks (4KB/partition contiguous).
            def xchunk(sb, h):
                nc.sync.dma_start(out=xt_sb[:, sb, 4 * h:4 * h + 4, :],
                                  in_=xP[:, sb, 4 * h:4 * h + 4, :])

            nc.sync.dma_start(
                out=w_sb["q"][:, :, :],
                in_=wqT.ap().rearrange("p (t e) -> p t e", t=NI))
            # first s-block in quarter chunks so projection it-tiles start
            # as soon as each 2-it slice lands
            for qtr in range(2):
                nc.sync.dma_start(out=xt_sb[:, 0, 2 * qtr:2 * qtr + 2, :],
                                  in_=xP[:, 0, 2 * qtr:2 * qtr + 2, :])
            nc.sync.dma_start(
                out=w_sb["k"][:, :, :],
                in_=wkT.ap().rearrange("p (t e) -> p t e", t=NI))
            for qtr in range(2, 4):
                nc.sync.dma_start(out=xt_sb[:, 0, 2 * qtr:2 * qtr + 2, :],
                                  in_=xP[:, 0, 2 * qtr:2 * qtr + 2, :])
            nc.sync.dma_start(out=bq_sb, in_=bq[:, :])
            nc.sync.dma_start(out=bk_sb, in_=bk[:, :])
            nc.sync.dma_start(out=bv_row, in_=bv[:, :])
            nc.sync.dma_start(out=mask_sb, in_=maskst[:, :])
            nc.sync.dma_start(out=id_sb, in_=ident[:, :])
            nc.sync.dma_start(
                out=w_sb["v"][:, :, :],
                in_=wvT.ap().rearrange("p (t e) -> p t e", t=NI))
            xchunk(1, 0)
            xchunk(1, 1)
            nc.sync.dma_start(out=wo_sb, in_=woT[:, :])
            for sb in range(2, S // 512):
                xchunk(sb, 0)
                xchunk(sb, 1)

            nc.gpsimd.partition_broadcast(bv_bc[:, :], bv_row[0:1, :])

            qt_sb = const.tile([128, S], BF16, tag="qt")
            kt_sb = const.tile([128, S], BF16, tag="kt")
            v_sb = const.tile([128, NKT, 130], BF16, tag="v")
            nc.vector.memset(v_sb[:, :, 64:65], 1.0)
            nc.vector.memset(v_sb[:, :, 129:130], 1.0)

            # PSUM banks: sc 2x2 + acc0/acc1 1x1 each + op 2x1 = 8
            with tc.tile_pool(name="ps", bufs=1, space="PSUM") as ps_pool, \
                 tc.tile_pool(name="spt", bufs=8) as spt, \
                 tc.tile_pool(name="satt", bufs=2) as satt, \
                 tc.tile_pool(name="satT", bufs=4) as satT, \
                 tc.tile_pool(name="srcp", bufs=4) as srcp, \
                 tc.tile_pool(name="sstage", bufs=4) as sstage:

                qk_emitted = [0]  # highest sb with q/k projection emitted
                qkproj_ps = {}

                def emit_qkproj_half(name, dst, bias, sb, half):
                    # half 0 emits its 0..3, half 1 its 4..7 + bias add, so
                    # score matmuls can interleave mid-projection and keep
                    # the exp stream fed
                    w = w_sb[name]
                    if half == 0:
                        qkproj_ps[(name, sb)] = ps_pool.tile(
                            [128, 512], F32, tag="op", bufs=2,
                            name=f"pj{name}{sb}")
                    ps = qkproj_ps[(name, sb)]
                    for it in range(4 * half, 4 * half + 4):
                        nc.tensor.matmul(
                            ps[:, 0:512],
                            lhsT=w[:, it, :],
                            rhs=xt_sb[:, sb, it, :],
                            start=(it == 0), stop=(it == NI - 1),
                        )
                    if half == 1:
                        nc.vector.tensor_scalar_add(
                            dst[:, sb * 512:(sb + 1) * 512], ps[:, 0:512],
                            bias[:, 0:1])
                        if name == "k":
                            qk_emitted[0] = max(qk_emitted[0], sb)

                def emit_qkproj_one(name, dst, bias, sb):
                    emit_qkproj_half(name, dst, bias, sb, 0)
                    emit_qkproj_half(name, dst, bias, sb, 1)

                wv = w_sb["v"]
                vproj_done = [0]

                def emit_vproj_one(st):
                    ps = ps_pool.tile([128, 512], F32, tag="op", bufs=2,
                                      name=f"pjv{st}")
                    for it in range(NI):
                        nc.tensor.matmul(
                            ps[:, 0:EC],
                            lhsT=xt_sb[:, st // 4, it,
                                       (st % 4) * 128:(st % 4) * 128 + 128],
                            rhs=wv[:, it, :],
                            start=(it == 0), stop=(it == NI - 1),
                        )
                    nc.vector.tensor_add(
                        v_sb[:, st, 0:64], ps[:, 0:64], bv_bc[:, 0:64])
                    nc.vector.tensor_add(
                        v_sb[:, st, 65:129], ps[:, 64:128], bv_bc[:, 64:128])

                # attn.V in transposed orientation: for each 128-wide q
                # subtile and head, acc_h[q, 0:65] += pT_h.T @ V'_h.
                # A start=True matmul zeroes the acc bank's WHOLE 2KB zero
                # region, so exactly one start (first matmul into the bank)
                # and one stop (last matmul, the qt=3 diagonal) per block --
                # the 4 packed q-subtile regions share the zeroing.
                def emit_attnv(acc, jpt, qb):
                    j, pt, off, r = jpt
                    nkt = 4 * (qb + 1)
                    for h in range(2):
                        for qt in range(max(r, 0), 4):
                            nc.tensor.matmul(
                                acc[h][:, qt * 65:qt * 65 + 65],
                                lhsT=pt[:, 512 * h + qt * 128 - off:
                                        512 * h + qt * 128 - off + 128],
                                rhs=v_sb[:, j, 65 * h:65 * h + 65],
                                start=(j == 0 and qt == max(r, 0)),
                                stop=(j == nkt - 1),
                                skip_group_check=True,
                            )

                # copy-engine rotation for PSUM drains (Pool-heavy; DVE help)
                drain_rr = [0]

                def drain_copy(dst, src, tail=False, qb=0):
                    # GPSIMD cannot read PSUM on hardware: drains live on DVE,
                    # with ScalarE helping while it still has exp slack
                    if tail:
                        engines = (nc.vector, nc.scalar)
                    else:
                        engines = (nc.vector,)
                    e = engines[drain_rr[0] % len(engines)]
                    drain_rr[0] += 1
                    if e is nc.scalar:
                        e.copy(dst, src)
                    else:
                        e.tensor_copy(dst, src)

                def emit_norm(qb, acc, att, rcp, qt=None, split=False):
                    # 1/denominator; qt=None does all 4 q-subtiles at once
                    qts = range(4) if qt is None else (qt,)
                    for h in range(2):
                        if qt is None:
                            a = acc[h][:, :]
                            den = bass.AP(tensor=a.tensor,
                                          offset=a.offset + 64,
                                          ap=[a.ap[0], [65, 4]])
                            nc.vector.reciprocal(rcp[:, 4 * h:4 * h + 4], den)
                        else:
                            nc.vector.reciprocal(
                                rcp[:, 4 * h + qt:4 * h + qt + 1],
                                acc[h][:, qt * 65 + 64:qt * 65 + 65])
                    for q in qts:
                        for h in range(2):
                            # split puts head 1 on ScalarE (exp-free in the
                            # endgame) so the tail transpose starts sooner
                            if split and h == 1:
                                nc.scalar.mul(
                                    att[:, q * 128 + 64 * h:
                                        q * 128 + 64 * h + 64],
                                    acc[h][:, q * 65:q * 65 + 64],
                                    rcp[:, 4 * h + q:4 * h + q + 1])
                            else:
                                nc.vector.tensor_scalar_mul(
                                    att[:, q * 128 + 64 * h:
                                        q * 128 + 64 * h + 64],
                                    acc[h][:, q * 65:q * 65 + 64],
                                    rcp[:, 4 * h + q:4 * h + q + 1])

                def emit_transpose(qb, att, attT, qt):
                    # PE transpose via identity: [128q, 128d] -> [128d, 128q]
                    trT = ps_pool.tile([128, 128], BF16, tag="op", bufs=2,
                                       name=f"tr{qb}_{qt}")
                    nc.tensor.transpose(trT[:, :],
                                        att[:, qt * 128:(qt + 1) * 128],
                                        id_sb[:, :])
                    nc.vector.tensor_copy(attT[:, qt * 128:(qt + 1) * 128],
                                          trT[:, :])

                def emit_oproj_one(qb, qt, nh, attT, stage, tail=False):
                    op = ps_pool.tile([128, 512], F32, tag="op", bufs=2,
                                      name=f"op{qb}_{qt}_{nh}")
                    nc.tensor.matmul(
                        op[:, :],
                        lhsT=attT[:, qt * 128:(qt + 1) * 128],
                        rhs=wo_sb[:, nh * 512:(nh + 1) * 512],
                        start=True, stop=True,
                    )
                    drain_copy(stage[:, qt, nh * 512:(nh + 1) * 512],
                               op[:, :], tail=tail, qb=qb)
                    if nh == 1:
                        nc.sync.dma_start(
                            out=out[qb * 512 + qt * 128:
                                    qb * 512 + (qt + 1) * 128, :],
                            in_=stage[:, qt, :])

                # global tile stream: (qb, j) in consumption order; the
                # scores->exp stage runs AHEAD tiles in front of the attn.V
                # stage so ScalarE saturates during the PE-heavy early blocks
                AHEAD = 32
                TILES = [(qb, j) for qb in range(NQB)
                         for j in range(4 * (qb + 1))]
                GIDX = {t: i for i, t in enumerate(TILES)}
                ptmap = {}
                cursor = [0]

                def emit_exp_tile(gi):
                    eqb, j = TILES[gi]
                    r = j - 4 * eqb  # >= 0 on the causal diagonal
                    off = 128 * r if r > 0 else 0
                    w = 512 - off   # valid q columns for this k-tile
                    sc = ps_pool.tile([128, 1024], F32, tag="sc", bufs=2,
                                      name=f"sc{eqb}_{j}")
                    for h in range(2):
                        hp = slice(64 * h, 64 * h + 64)
                        nc.tensor.matmul(
                            sc[:, 512 * h:512 * h + w],
                            lhsT=kt_sb[hp, j * 128:(j + 1) * 128],
                            rhs=qt_sb[hp, eqb * 512 + off:(eqb + 1) * 512],
                            start=True, stop=True,
                        )
                    if eqb == 1:
                        pt = spt.tile([128, 1024], BF16, tag="pt1", bufs=8,
                                      name=f"pt{eqb}_{j}")
                    else:
                        pt = spt.tile([128, 1024], BF16, tag="pt", bufs=26,
                                      name=f"pt{eqb}_{j}")
                    if r >= 0:
                        # one exp over both heads' [0:w] and [512:512+w]
                        # slices via a strided AP
                        def _two(t, w=w):
                            a = t[:, :]
                            return bass.AP(tensor=a.tensor, offset=a.offset,
                                           ap=[a.ap[0], [512, 2], [1, w]])
                        nc.scalar.activation(_two(pt), _two(sc), AF.Exp)
                        m = mask_sb[:, 384:384 + w]
                        for h in range(2):
                            pslc = pt[:, 512 * h:512 * h + w]
                            nc.vector.tensor_mul(pslc, pslc, m)
                    else:
                        nc.scalar.activation(pt[:, :], sc[:, :], AF.Exp)
                    ptmap[gi] = (j, pt, off, r)

                def advance_exp(upto):
                    while cursor[0] < min(upto, len(TILES)) and \
                            TILES[cursor[0]][0] <= qk_emitted[0]:
                        emit_exp_tile(cursor[0])
                        cursor[0] += 1

                # HAM warmup: cheap matmuls into the (not yet used) acc banks
                # while the first DMAs are in flight, so pe_busy_start lands
                # early and the real projections run at the warm clock.
                for i in range(12):
                    wp = ps_pool.tile([128, 260], F32, tag=f"acc{i % 2}",
                                      name=f"warm{i}")
                    nc.tensor.matmul(wp[:, :], lhsT=warm_src[:, 0:128],
                                     rhs=warm_src[:, :], start=True, stop=True)

                emit_qkproj_one("q", qt_sb, bq_sb, 0)
                emit_qkproj_one("k", kt_sb, bk_sb, 0)
                for st in range(4):
                    emit_vproj_one(st)
                vproj_done[0] = 4

                pending_epi = []   # prev-qb norm+transposes (must precede
                                   # this qb's first attn.V into acc)
                pending = []       # deferrable oproj items (1-2 qb backlog)

                qk_scheduled = [1]
                # block 1 is consumed last: its (early-computed) exps vacate
                # the ACT-bound endgame and its attn.V+epilogue give the tail
                # PE work while ScalarE drains
                BLOCK_ORDER = [0, 2, 3, 4, 5, 6, 7, 1]

                for pos, qb in enumerate(BLOCK_ORDER):
                    # bg items are CHAINS: multi-part chains keep their "op"
                    # psum tile across parts, so parts must be emitted with
                    # no other op-tag allocation in between
                    bg = []
                    for sb in range(qk_scheduled[0], min(qb + 3, NQB)):
                        for name, dst, bias in (("q", qt_sb, bq_sb),
                                                ("k", kt_sb, bk_sb)):
                            bg.append([
                                lambda n=name, d=dst, b=bias, s=sb, hf=hf:
                                emit_qkproj_half(n, d, b, s, hf)
                                for hf in range(2)])
                    qk_scheduled[0] = max(qk_scheduled[0], min(qb + 3, NQB))
                    nxt = BLOCK_ORDER[pos + 1] if pos + 1 < NQB else 0
                    lo = vproj_done[0]
                    hi = max(lo, 4 * (max(qb, nxt) + 1))
                    for st in range(lo, hi):
                        bg.append([lambda st=st: emit_vproj_one(st)])
                    vproj_done[0] = hi
                    chain = []

                    def pop_bg():
                        if not chain and bg:
                            chain.extend(bg.pop(0))
                        if chain:
                            chain.pop(0)()
                            return True
                        return False

                    nkt = 4 * (qb + 1)
                    last = pos == NQB - 1
                    tail_soon = pos == NQB - 2
                    acc = [ps_pool.tile([128, 260], F32, tag=f"acc{h}",
                                        name=f"acc{h}_{qb}")
                           for h in range(2)]
                    att = satt.tile([128, 512], BF16, tag="att",
                                    name=f"att{qb}")
                    attT = satT.tile([128, 512], BF16, tag="attT",
                                     name=f"attT{qb}")
                    rcp = srcp.tile([128, 8], F32, tag="rcp", name=f"rcp{qb}")
                    stage = sstage.tile([128, 4, E], BF16, tag="stage",
                                        name=f"stage{qb}")
                    reserve = 4 if last else (5 if qb >= 5 else 8)
                    for j in range(nkt):
                        gi = GIDX[(qb, j)]
                        advance_exp(gi + AHEAD)
                        if chain:
                            chain.pop(0)()            # finish open bg chain
                        elif j == 0 and pending_epi:
                            pending_epi.pop(0)()      # prev norm+transposes
                        elif j % 2 == 1 and bg:
                            pop_bg()                  # time-critical projs
                        elif len(pending) > reserve:
                            pending.pop(0)()          # prev oproj, one tile
                        else:
                            pop_bg()
                        if j == 3 and nkt <= 8:
                            pop_bg()                  # small blocks: drain bg
                        advance_exp(gi + AHEAD)
                        emit_attnv(acc, ptmap.pop(gi), qb)
                        if last and j >= 4 * qb:
                            # tail: per-q-subtile chains pipelined across
                            # engines right after the diagonal lands; spend
                            # the reserved oproj items in the norm latency
                            qt = j - 4 * qb
                            emit_norm(qb, acc, att, rcp, qt=qt,
                                      split=(qt >= 2))
                            if pending:
                                pending.pop(0)()
                            emit_transpose(qb, att, attT, qt)
                            for nh in range(2):
                                emit_oproj_one(qb, qt, nh, attT, stage,
                                               tail=True)
                    while chain or bg:
                        pop_bg()

                    if not last:
                        # cap the oproj backlog at one block so tile-pool
                        # buffer reuse can't order a writer before its reader
                        while len(pending) > 24:
                            pending.pop(0)()

                        def epi(qb=qb, acc=acc, att=att, attT=attT,
                                rcp=rcp, sp=tail_soon):
                            emit_norm(qb, acc, att, rcp, split=sp)
                            for qt in range(4):
                                emit_transpose(qb, att, attT, qt)
                        pending_epi.append(epi)
                        for qt in range(4):
                            for nh in range(2):
                                pending.append(
                                    lambda qb=qb, qt=qt, nh=nh, a=attT,
                                    s=stage, tl=tail_soon:
                                    emit_oproj_one(qb, qt, nh, a, s, tail=tl))

    nc.compile()
    return nc


def _make_mask_strip():
    k = np.arange(128)[:, None]
    t = np.arange(896)[None, :]
    return (k <= t - 384).astype(np.float32)


def _pack_w(wT):
    # [E, EC] -> [128, NI*EC] with packed[p, it*EC+e] = wT[it*128+p, e]
    E, EC = wT.shape
    return np.ascontiguousarray(
        wT.reshape(E // 128, 128, EC).transpose(1, 0, 2).reshape(128, -1))


def _shard_inputs(x, Wq, bq, Wk, bk, Wv, bv, Wo):
    import ml_dtypes
    bf16 = ml_dtypes.bfloat16
    S, E = x.shape[-2], x.shape[-1]
    xP = np.ascontiguousarray(
        np.asarray(x, np.float32).reshape(S // 512, 512, E // 128, 128)
        .transpose(3, 0, 2, 1)).astype(bf16)
    strip = _make_mask_strip().astype(bf16)
    eye = np.eye(128, dtype=np.float32).astype(bf16)
    in_maps = []
    for c in range(N_CORES):
        sl = slice(128 * c, 128 * (c + 1))
        in_maps.append({
            "xP": xP,
            "wqT": _pack_w((np.asarray(Wq, np.float32)[sl, :] / 8.0).T).astype(bf16),
            "wkT": _pack_w(np.asarray(Wk, np.float32)[sl, :].T).astype(bf16),
            "wvT": _pack_w(np.asarray(Wv, np.float32)[sl, :].T).astype(bf16),
            "woT": np.ascontiguousarray(np.asarray(Wo, np.float32)[:, sl].T).astype(bf16),
            "bq": (np.asarray(bq, np.float32)[sl] / 8.0).reshape(128, 1),
            "bk": np.asarray(bk, np.float32)[sl].reshape(128, 1),
            "bv": np.asarray(bv, np.float32)[sl].reshape(1, 128),
            "maskst": strip,
            "ident": eye,
        })
    return in_maps


_NC_CACHE = {}


def kernel(x, Wq, bq, Wk, bk, Wv, bv, Wo, bo):
    x = np.asarray(x)
    B, S, E = x.shape
    if (S, E) not in _NC_CACHE:
        _NC_CACHE[(S, E)] = _build_nc(S=S, E=E)
    nc = _NC_CACHE[(S, E)]

    in_maps = _shard_inputs(x, Wq, bq, Wk, bk, Wv, bv, Wo)
    res = run_bass_kernel_spmd(nc, in_maps, list(range(N_CORES)))

    total = np.zeros((S, E), np.float32)
    for r in res.results:
        total += np.asarray(r["out"], np.float32).reshape(S, E)
    total += np.asarray(bo, np.float32)
    return total.reshape(B, S, E).astype(np.float32)


# revision 110
# speedup vs baseline: 1.1425x; 1.0126x over previous
"""Causal multi-head attention (B=1, S=4096, E=1024, H=16, Dk=64) on 8 TRN2
NeuronCores via Bass/Tile, head-sharded (tensor parallel): core c computes
heads 2c and 2c+1 end-to-end plus its partial output projection; the host sums
the 8 partials (bf16) and adds the output bias.

Per-core program (transposed attn.V + global exp-ahead pipeline):
  QT/KT[e'=128, S] = (W x^T + b) in bf16 (softmax 1/sqrt(Dk) folded into Wq/bq)
  V'[k, 2*65]      = x Wv^T + bv, with a ones column per head
  global tile stream, scores->exp running AHEAD tiles in front of attn.V
  consumption so ScalarE (the co-bottleneck at ~141us busy, vs ~143us on PE; 169,959ns total)
  stays saturated through the PE-heavy projection phase:
    scoresT[k, q] via PE (2 heads packed with row tiling, d=64 each)
    pT = exp(scoresT) on ScalarE (no max subtraction; scores are ~N(0,1))
    diagonal tiles: multiply by causal 0/1 mask strip (post-exp)
    per q-subtile (128) and head: accT_h[q, 0:65] += pT_h.T @ V'_h
      (out partitions = 128 q values -> half the PE cycles of the
       [65, q] orientation; column 64 accumulates the softmax denom;
       the 4 q-subtile regions share one PSUM bank, so exactly one
       start/stop pair per bank per block -- start zeroes the whole
       2KB zero region)
  att[q, d] = accT[q, 0:64] * (1/accT[q, 64])   (per-partition scalar on DVE)
  attT[d, q] via PE transpose (identity matmul), then
  partial[q, e] = attT.T @ Wo_c.T ; drained to bf16 partial output.
  Output-projection work is deferred (a standing ~8-16 item backlog) into
  the exp-bound late blocks to fill PE; PSUM drains live on DVE only
  (GPSIMD cannot read PSUM), with ScalarE helping once exps are done.
"""

import numpy as np

import concourse.bass as bass
import concourse.mybir as mybir
import concourse.tile as tile
from concourse import bacc
from concourse.bass_utils import run_bass_kernel_spmd

F32 = mybir.dt.float32
BF16 = mybir.dt.bfloat16
AF = mybir.ActivationFunctionType

EMBED_DIM = 1024
NUM_HEADS = 16
SEQ = 4096
BATCH = 1
N_CORES = 8


def _build_nc(S=SEQ, E=EMBED_DIM):
    EC = 128          # per-core feature slice (2 heads x 64)
    NI = E // 128     # contraction tiles for projections
    NQB = S // 512    # q blocks
    NKT = S // 128    # k tiles

    nc = bacc.Bacc(None, target_bir_lowering=False, debug=False)

    # x arrives pre-permuted to the SBUF layout: xP[p, sb, it, s'] =
    # x[sb*512+s', it*128+p] -- one contiguous 8KB line per partition per
    # 512-column s-block (full-rate DMA, no mid-dim segmentation)
    xP = nc.dram_tensor("xP", [128, S // 512, E // 128, 512], BF16,
                        kind="ExternalInput")
    # projection weights arrive pre-packed as [128, NI*EC]:
    # packed[p, it*EC + e] = W.T[it*128 + p, e]  (contiguous DMA lines)
    wqT = nc.dram_tensor("wqT", [128, NI * EC], BF16, kind="ExternalInput")
    wkT = nc.dram_tensor("wkT", [128, NI * EC], BF16, kind="ExternalInput")
    wvT = nc.dram_tensor("wvT", [128, NI * EC], BF16, kind="ExternalInput")
    woT = nc.dram_tensor("woT", [EC, E], BF16, kind="ExternalInput")
    bq = nc.dram_tensor("bq", [EC, 1], F32, kind="ExternalInput")
    bk = nc.dram_tensor("bk", [EC, 1], F32, kind="ExternalInput")
    bv = nc.dram_tensor("bv", [1, EC], F32, kind="ExternalInput")
    maskst = nc.dram_tensor("maskst", [128, 896], BF16, kind="ExternalInput")
    ident = nc.dram_tensor("ident", [128, 128], BF16, kind="ExternalInput")
    out = nc.dram_tensor("out", [S, E], BF16, kind="ExternalOutput")

    with tile.TileContext(nc) as tc:
        with tc.tile_pool(name="const", bufs=1) as const:
            w_sb = {}
            for name in ("q", "k", "v"):
                w_sb[name] = const.tile([128, NI, EC], BF16, tag=f"w{name}",
                                        name=f"w{name}")
            xt_sb = const.tile([128, S // 512, NI, 512], BF16, tag="xt")
            bq_sb = const.tile([128, 1], F32, tag="bq")
            bk_sb = const.tile([128, 1], F32, tag="bk")
            bv_row = const.tile([1, EC], F32, tag="bvr")
            bv_bc = const.tile([128, EC], F32, tag="bv")
            mask_sb = const.tile([128, 896], BF16, tag="mask")
            wo_sb = const.tile([128, E], BF16, tag="wo")
            id_sb = const.tile([128, 128], BF16, tag="ident")
            warm_src = const.tile([128, 260], BF16, tag="warmsrc")
            warm_act = const.tile([128, 1], BF16, tag="warmact")
            nc.vector.memset(warm_src[:, :], 1.0)
            # preload the Exp activation table off the critical path
            nc.scalar.activation(warm_act[:, :], warm_src[:, 0:1], AF.Exp)

            # DMA issue order = arrival order: q weights, first x half-chunk,
            # k weights, ... so the first projection can start ~4.5us in.
            # x streams in half-s-block chunks (4KB/partition contiguous).
            def xchunk(sb, h):
                nc.sync.dma_start(out=xt_sb[:, sb, 4 * h:4 * h + 4, :],
                                  in_=xP[:, sb, 4 * h:4 * h + 4, :])

            nc.sync.dma_start(
                out=w_sb["q"][:, :, :],
                in_=wqT.ap().rearrange("p (t e) -> p t e", t=NI))
            # first s-block in quarter chunks so projection it-tiles start
            # as soon as each 2-it slice lands
            for qtr in range(2):
                nc.sync.dma_start(out=xt_sb[:, 0, 2 * qtr:2 * qtr + 2, :],
                                  in_=xP[:, 0, 2 * qtr:2 * qtr + 2, :])
            nc.sync.dma_start(
                out=w_sb["k"][:, :, :],
                in_=wkT.ap().rearrange("p (t e) -> p t e", t=NI))
            for qtr in range(2, 4):
                nc.sync.dma_start(out=xt_sb[:, 0, 2 * qtr:2 * qtr + 2, :],
                                  in_=xP[:, 0, 2 * qtr:2 * qtr + 2, :])
            nc.sync.dma_start(out=bq_sb, in_=bq[:, :])
            nc.sync.dma_start(out=bk_sb, in_=bk[:, :])
            nc.sync.dma_start(out=bv_row, in_=bv[:, :])
            nc.sync.dma_start(out=mask_sb, in_=maskst[:, :])
            nc.sync.dma_start(out=id_sb, in_=ident[:, :])
            nc.sync.dma_start(
                out=w_sb["v"][:, :, :],
                in_=wvT.ap().rearrange("p (t e) -> p t e", t=NI))
            xchunk(1, 0)
            xchunk(1, 1)
            nc.sync.dma_start(out=wo_sb, in_=woT[:, :])
            for sb in range(2, S // 512):
                xchunk(sb, 0)
                xchunk(sb, 1)

            nc.gpsimd.partition_broadcast(bv_bc[:, :], bv_row[0:1, :])

            qt_sb = const.tile([128, S], BF16, tag="qt")
            kt_sb = const.tile([128, S], BF16, tag="kt")
            v_sb = const.tile([128, NKT, 130], BF16, tag="v")
            nc.vector.memset(v_sb[:, :, 64:65], 1.0)
            nc.vector.memset(v_sb[:, :, 129:130], 1.0)

            # PSUM banks: sc 2x2 + acc0/acc1 1x1 each + op 2x1 = 8
            with tc.tile_pool(name="ps", bufs=1, space="PSUM") as ps_pool, \
                 tc.tile_pool(name="spt", bufs=8) as spt, \
                 tc.tile_pool(name="satt", bufs=2) as satt, \
                 tc.tile_pool(name="satT", bufs=4) as satT, \
                 tc.tile_pool(name="srcp", bufs=4) as srcp, \
                 tc.tile_pool(name="sstage", bufs=4) as sstage:

                qk_emitted = [0]  # highest sb with q/k projection emitted
                qkproj_ps = {}

                def emit_qkproj_half(name, dst, bias, sb, half):
                    # half 0 emits its 0..3, half 1 its 4..7 + bias add, so
                    # score matmuls can interleave mid-projection and keep
                    # the exp stream fed
                    w = w_sb[name]
                    if half == 0:
                        qkproj_ps[(name, sb)] = ps_pool.tile(
                            [128, 512], F32, tag="op", bufs=2,
                            name=f"pj{name}{sb}")
                    ps = qkproj_ps[(name, sb)]
                    for it in range(4 * half, 4 * half + 4):
                        nc.tensor.matmul(
                            ps[:, 0:512],
                            lhsT=w[:, it, :],
                            rhs=xt_sb[:, sb, it, :],
                            start=(it == 0), stop=(it == NI - 1),
                        )
                    if half == 1:
                        nc.vector.tensor_scalar_add(
                            dst[:, sb * 512:(sb + 1) * 512], ps[:, 0:512],
                            bias[:, 0:1])
                        if name == "k":
                            qk_emitted[0] = max(qk_emitted[0], sb)

                def emit_qkproj_one(name, dst, bias, sb):
                    emit_qkproj_half(name, dst, bias, sb, 0)
                    emit_qkproj_half(name, dst, bias, sb, 1)

                wv = w_sb["v"]
                vproj_done = [0]

                def emit_vproj_one(st):
                    ps = ps_pool.tile([128, 512], F32, tag="op", bufs=2,
                                      name=f"pjv{st}")
                    for it in range(NI):
                        nc.tensor.matmul(
                            ps[:, 0:EC],
                            lhsT=xt_sb[:, st // 4, it,
                                       (st % 4) * 128:(st % 4) * 128 + 128],
                            rhs=wv[:, it, :],
                            start=(it == 0), stop=(it == NI - 1),
                        )
                    nc.vector.tensor_add(
                        v_sb[:, st, 0:64], ps[:, 0:64], bv_bc[:, 0:64])
                    nc.vector.tensor_add(
                        v_sb[:, st, 65:129], ps[:, 64:128], bv_bc[:, 64:128])

                # attn.V in transposed orientation: for each 128-wide q
                # subtile and head, acc_h[q, 0:65] += pT_h.T @ V'_h.
                # A start=True matmul zeroes the acc bank's WHOLE 2KB zero
                # region, so exactly one start (first matmul into the bank)
                # and one stop (last matmul, the qt=3 diagonal) per block --
                # the 4 packed q-subtile regions share the zeroing.
                def emit_attnv(acc, jpt, qb):
                    j, pt, off, r = jpt
                    nkt = 4 * (qb + 1)
                    for h in range(2):
                        for qt in range(max(r, 0), 4):
                            nc.tensor.matmul(
                                acc[h][:, qt * 65:qt * 65 + 65],
                                lhsT=pt[:, 512 * h + qt * 128 - off:
                                        512 * h + qt * 128 - off + 128],
                                rhs=v_sb[:, j, 65 * h:65 * h + 65],
                                start=(j == 0 and qt == max(r, 0)),
                                stop=(j == nkt - 1),
                                skip_group_check=True,
                            )

                # copy-engine rotation for PSUM drains (Pool-heavy; DVE help)
                drain_rr = [0]

                def drain_copy(dst, src, tail=False, qb=0):
                    # GPSIMD cannot read PSUM on hardware: drains live on DVE,
                    # with ScalarE helping while it still has exp slack
                    if tail:
                        engines = (nc.vector, nc.scalar)
                    else:
                        engines = (nc.vector,)
                    e = engines[drain_rr[0] % len(engines)]
                    drain_rr[0] += 1
                    if e is nc.scalar:
                        e.copy(dst, src)
                    else:
                        e.tensor_copy(dst, src)

                def emit_norm(qb, acc, att, rcp, qt=None, split=False):
                    # 1/denominator; qt=None does all 4 q-subtiles at once
                    qts = range(4) if qt is None else (qt,)
                    for h in range(2):
                        if qt is None:
                            a = acc[h][:, :]
                            den = bass.AP(tensor=a.tensor,
                                          offset=a.offset + 64,
                                          ap=[a.ap[0], [65, 4]])
                            nc.vector.reciprocal(rcp[:, 4 * h:4 * h + 4], den)
                        else:
                            nc.vector.reciprocal(
                                rcp[:, 4 * h + qt:4 * h + qt + 1],
                                acc[h][:, qt * 65 + 64:qt * 65 + 65])
                    for q in qts:
                        for h in range(2):
                            # split puts head 1 on ScalarE (exp-free in the
                            # endgame) so the tail transpose starts sooner
                            if split and h == 1:
                                nc.scalar.mul(
                                    att[:, q * 128 + 64 * h:
                                        q * 128 + 64 * h + 64],
                                    acc[h][:, q * 65:q * 65 + 64],
                                    rcp[:, 4 * h + q:4 * h + q + 1])
                            else:
                                nc.vector.tensor_scalar_mul(
                                    att[:, q * 128 + 64 * h:
                                        q * 128 + 64 * h + 64],
                                    acc[h][:, q * 65:q * 65 + 64],
                                    rcp[:, 4 * h + q:4 * h + q + 1])

                def emit_transpose(qb, att, attT, qt):
                    # PE transpose via identity: [128q, 128d] -> [128d, 128q]
                    trT = ps_pool.tile([128, 128], BF16, tag="op", bufs=2,
                                       name=f"tr{qb}_{qt}")
                    nc.tensor.transpose(trT[:, :],
                                        att[:, qt * 128:(qt + 1) * 128],
                                        id_sb[:, :])
                    nc.vector.tensor_copy(attT[:, qt * 128:(qt + 1) * 128],
                                          trT[:, :])

                def emit_oproj_one(qb, qt, nh, attT, stage, tail=False):
                    op = ps_pool.tile([128, 512], F32, tag="op", bufs=2,
                                      name=f"op{qb}_{qt}_{nh}")
                    nc.tensor.matmul(
                        op[:, :],
                        lhsT=attT[:, qt * 128:(qt + 1) * 128],
                        rhs=wo_sb[:, nh * 512:(nh + 1) * 512],
                        start=True, stop=True,
                    )
                    drain_copy(stage[:, qt, nh * 512:(nh + 1) * 512],
                               op[:, :], tail=tail, qb=qb)
                    if nh == 1:
                        nc.sync.dma_start(
                            out=out[qb * 512 + qt * 128:
                                    qb * 512 + (qt + 1) * 128, :],
                            in_=stage[:, qt, :])

                # global tile stream: (qb, j) in consumption order; the
                # scores->exp stage runs AHEAD tiles in front of the attn.V
                # stage so ScalarE saturates during the PE-heavy early blocks
                AHEAD = 32
                TILES = [(qb, j) for qb in range(NQB)
                         for j in range(4 * (qb + 1))]
                GIDX = {t: i for i, t in enumerate(TILES)}
                ptmap = {}
                cursor = [0]

                def emit_exp_tile(gi):
                    eqb, j = TILES[gi]
                    r = j - 4 * eqb  # >= 0 on the causal diagonal
                    off = 128 * r if r > 0 else 0
                    w = 512 - off   # valid q columns for this k-tile
                    sc = ps_pool.tile([128, 1024], F32, tag="sc", bufs=2,
                                      name=f"sc{eqb}_{j}")
                    for h in range(2):
                        hp = slice(64 * h, 64 * h + 64)
                        nc.tensor.matmul(
                            sc[:, 512 * h:512 * h + w],
                            lhsT=kt_sb[hp, j * 128:(j + 1) * 128],
                            rhs=qt_sb[hp, eqb * 512 + off:(eqb + 1) * 512],
                            start=True, stop=True,
                        )
                    if eqb == 1:
                        pt = spt.tile([128, 1024], BF16, tag="pt1", bufs=8,
                                      name=f"pt{eqb}_{j}")
                    else:
                        pt = spt.tile([128, 1024], BF16, tag="pt", bufs=26,
                                      name=f"pt{eqb}_{j}")
                    if r >= 0:
                        # one exp over both heads' [0:w] and [512:512+w]
                        # slices via a strided AP
                        def _two(t, w=w):
                            a = t[:, :]
                            return bass.AP(tensor=a.tensor, offset=a.offset,
                                           ap=[a.ap[0], [512, 2], [1, w]])
                        nc.scalar.activation(_two(pt), _two(sc), AF.Exp)
                        m = mask_sb[:, 384:384 + w]
                        for h in range(2):
                            pslc = pt[:, 512 * h:512 * h + w]
                            nc.vector.tensor_mul(pslc, pslc, m)
                    else:
                        nc.scalar.activation(pt[:, :], sc[:, :], AF.Exp)
                    ptmap[gi] = (j, pt, off, r)

                def advance_exp(upto):
                    while cursor[0] < min(upto, len(TILES)) and \
                            TILES[cursor[0]][0] <= qk_emitted[0]:
                        emit_exp_tile(cursor[0])
                        cursor[0] += 1

                # HAM warmup: cheap matmuls into the (not yet used) acc banks
                # while the first DMAs are in flight, so pe_busy_start lands
                # early and the real projections run at the warm clock.
                for i in range(12):
                    wp = ps_pool.tile([128, 260], F32, tag=f"acc{i % 2}",
                                      name=f"warm{i}")
                    nc.tensor.matmul(wp[:, :], lhsT=warm_src[:, 0:128],
                                     rhs=warm_src[:, :], start=True, stop=True)

                emit_qkproj_one("q", qt_sb, bq_sb, 0)
                emit_qkproj_one("k", kt_sb, bk_sb, 0)
                for st in range(4):
                    emit_vproj_one(st)
                vproj_done[0] = 4

                pending_epi = []   # prev-qb norm+transposes (must precede
                                   # this qb's first attn.V into acc)
                pending = []       # deferrable oproj items (1-2 qb backlog)

                qk_scheduled = [1]
                # block 1 is consumed last: its (early-computed) exps vacate
                # the ACT-bound endgame and its attn.V+epilogue give the tail
                # PE work while ScalarE drains
                BLOCK_ORDER = [0, 2, 3, 4, 5, 6, 7, 1]

                for pos, qb in enumerate(BLOCK_ORDER):
                    # bg items are CHAINS: multi-part chains keep their "op"
                    # psum tile across parts, so parts must be emitted with
                    # no other op-tag allocation in between
                    bg = []
                    for sb in range(qk_scheduled[0], min(qb + 3, NQB)):
                        for name, dst, bias in (("q", qt_sb, bq_sb),
                                                ("k", kt_sb, bk_sb)):
                            bg.append([
                                lambda n=name, d=dst, b=bias, s=sb, hf=hf:
                                emit_qkproj_half(n, d, b, s, hf)
                                for hf in range(2)])
                    qk_scheduled[0] = max(qk_scheduled[0], min(qb + 3, NQB))
                    nxt = BLOCK_ORDER[pos + 1] if pos + 1 < NQB else 0
                    lo = vproj_done[0]
                    hi = max(lo, 4 * (max(qb, nxt) + 1))
                    for st in range(lo, hi):
                        bg.append([lambda st=st: emit_vproj_one(st)])
                    vproj_done[0] = hi
                    chain = []

                    def pop_bg():
                        if not chain and bg:
                            chain.extend(bg.pop(0))
                        if chain:
                            chain.pop(0)()
                            return True
                        return False

                    nkt = 4 * (qb + 1)
                    last = pos == NQB - 1
                    tail_soon = pos == NQB - 2
                    acc = [ps_pool.tile([128, 260], F32, tag=f"acc{h}",
                                        name=f"acc{h}_{qb}")
                           for h in range(2)]
                    att = satt.tile([128, 512], BF16, tag="att",
                                    name=f"att{qb}")
                    attT = satT.tile([128, 512], BF16, tag="attT",
                                     name=f"attT{qb}")
                    rcp = srcp.tile([128, 8], F32, tag="rcp", name=f"rcp{qb}")
                    stage = sstage.tile([128, 4, E], BF16, tag="stage",
                                        name=f"stage{qb}")
                    reserve = 0 if last else (7 if qb >= 5 else 8)
                    for j in range(nkt):
                        gi = GIDX[(qb, j)]
                        advance_exp(gi + AHEAD)
                        if chain:
                            chain.pop(0)()            # finish open bg chain
                        elif j == 0 and pending_epi:
                            pending_epi.pop(0)()      # prev norm+transposes
                        elif j % 2 == 1 and bg:
                            pop_bg()                  # time-critical projs
                        elif len(pending) > reserve:
                            pending.pop(0)()          # prev oproj, one tile
                        else:
                            pop_bg()
                        if j == 3 and nkt <= 8:
                            pop_bg()                  # small blocks: drain bg
                        advance_exp(gi + AHEAD)
                        emit_attnv(acc, ptmap.pop(gi), qb)
                        if last and j >= 4 * qb:
                            # tail: per-q-subtile chains pipelined across
                            # engines right after the diagonal lands; spend
                            # the reserved oproj items in the norm latency
                            qt = j - 4 * qb
                            emit_norm(qb, acc, att, rcp, qt=qt,
                                      split=(qt >= 2))
                            if pending:
                                pending.pop(0)()
                            emit_transpose(qb, att, attT, qt)
                            for nh in range(2):
                                emit_oproj_one(qb, qt, nh, attT, stage,
                                               tail=True)
                    while chain or bg:
                        pop_bg()
                        # keep the exp stream fed through the end-of-block
                        # drain: qkproj chains completing here raise
                        # qk_emitted, unlocking the next blocks' tiles
                        advance_exp(GIDX[(qb, nkt - 1)] + AHEAD)

                    if not last:
                        # cap the oproj backlog at one block so tile-pool
                        # buffer reuse can't order a writer before its reader
                        while len(pending) > 24:
                            pending.pop(0)()

                        def epi(qb=qb, acc=acc, att=att, attT=attT,
                                rcp=rcp, sp=tail_soon):
                            emit_norm(qb, acc, att, rcp, split=sp)
                            for qt in range(4):
                                emit_transpose(qb, att, attT, qt)
                        pending_epi.append(epi)
                        for qt in range(4):
                            for nh in range(2):
                                pending.append(
                                    lambda qb=qb, qt=qt, nh=nh, a=attT,
                                    s=stage, tl=tail_soon:
                                    emit_oproj_one(qb, qt, nh, a, s, tail=tl))

    nc.compile()
    return nc


def _make_mask_strip():
    k = np.arange(128)[:, None]
    t = np.arange(896)[None, :]
    return (k <= t - 384).astype(np.float32)


def _pack_w(wT):
    # [E, EC] -> [128, NI*EC] with packed[p, it*EC+e] = wT[it*128+p, e]
    E, EC = wT.shape
    return np.ascontiguousarray(
        wT.reshape(E // 128, 128, EC).transpose(1, 0, 2).reshape(128, -1))


def _shard_inputs(x, Wq, bq, Wk, bk, Wv, bv, Wo):
    import ml_dtypes
    bf16 = ml_dtypes.bfloat16
    S, E = x.shape[-2], x.shape[-1]
    xP = np.ascontiguousarray(
        np.asarray(x, np.float32).reshape(S // 512, 512, E // 128, 128)
        .transpose(3, 0, 2, 1)).astype(bf16)
    strip = _make_mask_strip().astype(bf16)
    eye = np.eye(128, dtype=np.float32).astype(bf16)
    in_maps = []
    for c in range(N_CORES):
        sl = slice(128 * c, 128 * (c + 1))
        in_maps.append({
            "xP": xP,
            "wqT": _pack_w((np.asarray(Wq, np.float32)[sl, :] / 8.0).T).astype(bf16),
            "wkT": _pack_w(np.asarray(Wk, np.float32)[sl, :].T).astype(bf16),
            "wvT": _pack_w(np.asarray(Wv, np.float32)[sl, :].T).astype(bf16),
            "woT": np.ascontiguousarray(np.asarray(Wo, np.float32)[:, sl].T).astype(bf16),
            "bq": (np.asarray(bq, np.float32)[sl] / 8.0).reshape(128, 1),
            "bk": np.asarray(bk, np.float32)[sl].reshape(128, 1),
            "bv": np.asarray(bv, np.float32)[sl].reshape(1, 128),
            "maskst": strip,
            "ident": eye,
        })
    return in_maps


_NC_CACHE = {}


def kernel(x, Wq, bq, Wk, bk, Wv, bv, Wo, bo):
    x = np.asarray(x)
    B, S, E = x.shape
    if (S, E) not in _NC_CACHE:
        _NC_CACHE[(S, E)] = _build_nc(S=S, E=E)
    nc = _NC_CACHE[(S, E)]

    in_maps = _shard_inputs(x, Wq, bq, Wk, bk, Wv, bv, Wo)
    res = run_bass_kernel_spmd(nc, in_maps, list(range(N_CORES)))

    total = np.zeros((S, E), np.float32)
    for r in res.results:
        total += np.asarray(r["out"], np.float32).reshape(S, E)
    total += np.asarray(bo, np.float32)
    return total.reshape(B, S, E).astype(np.float32)
